# revision 26
# baseline (speedup 1.0000x reference)
"""Cross-attention Bass kernel for 8 trn2 NeuronCores — v3.

Sharding: core d handles batch b = d//4, query rows [(d%4)*1024, ...+1024),
all 8 heads (no collectives). Context compacted on host via mask, padded to
m_pad = ceil(max_meff/128)*128 (seed-0 inputs: 2056 -> 2176, 17 k-tiles).

v3 strategy (vs v2 baseline):
- Scores on the PE in fp8e4m3 with MatmulPerfMode.DoubleRow (0.5 cyc/row in
  the cost model vs 1.0 bf16): contraction D=64 packed as [32 partitions, 2]
  pairs. Wq/Wk columns are permuted ON THE HOST so the Q/K projection PSUM
  drains land in the DoubleRow-paired layout with partition-identity casts
  (no repartition pass): column (dc*128+p) holds head h=4*(dc//2)+(p//32),
  dim d=32*(dc%2)+(p%32). Scores stationary = kT8[32b:32b+32, hi, :, mtile],
  moving = qT8[32b:32b+32, hi, :, qblock].
- softmax scale (1/8) folded into ACT exp via activation(scale=...), so q/k
  keep natural magnitude in fp8 (no subnormal loss).
- PV unchanged (bf16, natural orientation, 65th valid column accumulates the
  denominator).
- Normalize batched per unit: Pool (gpsimd) drains each PV chain [128,65]
  f32 PSUM->SBUF stage; DVE then does ONE reciprocal [128,4,1] and ONE
  broadcast-mul [128,4,64] -> onat. v2 did per-chain recip+mul on DVE.
- v2 valid-column copies replaced by two memsets (compacted mask is
  [1]*m_eff + [0]*pad).
- Optional exp offload (dve_units): P = (t^2+0.5)^16 with
  t = s*SCALE/(16*sqrt2) + 1/sqrt2 == (1+a+a^2/2)^16, a = s*SCALE/16
  (rel err ~0.1% at |s*SCALE|=1.3). Pass 1 on Pool per score group (PSUM f32
  -> fp16), squarings on DVE in fp16 half-unit buffers, final mul writes pb
  bf16. Off by default; enabled when ACT is the critical engine.
"""
import numpy as np

B, N, M = 2, 4096, 4096
QUERY_DIM, CONTEXT_DIM = 512, 768
H, D = 8, 64
INNER = H * D  # 512
NCORES = 8
N_DEV = (B * N) // NCORES  # 1024 query rows per core
QB = 512
NQB = N_DEV // QB  # 2
SCALE = float(D) ** -0.5  # 0.125
M_PAD_MIN = 128

# exp offload constants: t = ALPHA*s_raw + BETA, P = (t*t+0.5)^16
ALPHA = SCALE / (16.0 * np.sqrt(2.0))
BETA = float(1.0 / np.sqrt(2.0))

DVE_UNITS = ()  # unit indices whose exp runs on Pool+DVE instead of ACT

_compiled = {}


def _perm():
    """Column permutation for Wq/Wk making projection drains land in the
    DoubleRow-paired fp8 layout."""
    perm = np.empty(INNER, dtype=np.int64)
    for dc in range(4):
        hi, i = dc // 2, dc % 2
        for p in range(128):
            b4, p5 = p // 32, p % 32
            h = 4 * hi + b4
            d = 32 * i + p5
            perm[dc * 128 + p] = h * 64 + d
    return perm


def _build(m_pad, dve_units=DVE_UNITS):
    from concourse import bacc
    import concourse.bass as bass
    import concourse.mybir as mybir
    import concourse.tile as tile

    F32 = mybir.dt.float32
    BF = mybir.dt.bfloat16
    FP16 = mybir.dt.float16
    F8 = mybir.dt.float8e4
    AF = mybir.ActivationFunctionType
    DRM = mybir.MatmulPerfMode.DoubleRow

    T = m_pad // 128  # k-tiles
    if dve_units:
        PBUFS = 5 if T <= 17 else (4 if T <= 19 else 3)
    else:
        PBUFS = 6 if T <= 17 else (5 if T <= 19 else 3)
    MBLK = [(s, min(512, m_pad - s)) for s in range(0, m_pad, 512)]
    SC_G = 3
    GROUPS = [(g, min(SC_G, T - g)) for g in range(0, T, SC_G)]
    # tile ranges for the two half-unit exp-offload squaring passes
    HALF = [(0, (T + 1) // 2), ((T + 1) // 2, T)]
    TH = max(h1 - h0 for h0, h1 in HALF)

    CQ = QUERY_DIM // 128  # 4
    CC = CONTEXT_DIM // 128  # 6
    CI = INNER // 128  # 4

    nc = bacc.Bacc()
    xs_d = nc.declare_dram_parameter("xs", [N_DEV, QUERY_DIM], BF, isOutput=False)
    ctx_d = nc.declare_dram_parameter("ctx", [m_pad, CONTEXT_DIM], BF, isOutput=False)
    val_d = nc.declare_dram_parameter("valid", [m_pad], BF, isOutput=False)
    wq_d = nc.declare_dram_parameter("Wq", [QUERY_DIM, INNER], BF, isOutput=False)
    wk_d = nc.declare_dram_parameter("Wk", [CONTEXT_DIM, INNER], BF, isOutput=False)
    wv_d = nc.declare_dram_parameter("Wv", [CONTEXT_DIM, INNER], BF, isOutput=False)
    wo_d = nc.declare_dram_parameter("Wo", [INNER, QUERY_DIM], BF, isOutput=False)
    bo_d = nc.declare_dram_parameter("bo", [QUERY_DIM], F32, isOutput=False)
    out_d = nc.declare_dram_parameter("out", [N_DEV, QUERY_DIM], F32, isOutput=True)
    dbg = getattr(_build, "debug", False)
    if dbg:
        dq_d = nc.declare_dram_parameter("dbg_qT8", [128, 2, 2, N_DEV], F8, isOutput=True)
        dk_d = nc.declare_dram_parameter("dbg_kT8", [128, 2, 2, m_pad], F8, isOutput=True)
        dv_d = nc.declare_dram_parameter("dbg_v2", [128, T, H, 65], BF, isOutput=True)
        don_d = nc.declare_dram_parameter("dbg_onat0", [128, CI, H, 64], BF, isOutput=True)
        dot_d = nc.declare_dram_parameter("dbg_oT0", [128, CI, QB], BF, isOutput=True)
        don1_d = nc.declare_dram_parameter("dbg_onat1", [128, CI, H, 64], BF, isOutput=True)
        dot1_d = nc.declare_dram_parameter("dbg_oT1", [128, CI, QB], BF, isOutput=True)
        dpb_d = nc.declare_dram_parameter("dbg_pb15", [128, T, 512], BF, isOutput=True)

    with tile.TileContext(nc) as tc:
        with (
            tc.tile_pool(name="big", bufs=1) as big,
            tc.tile_pool(name="ctxt", bufs=2) as ctxt,
            tc.tile_pool(name="pb", bufs=PBUFS) as pbp,
            tc.tile_pool(name="tay", bufs=2) as tay,
            tc.tile_pool(name="sm", bufs=4) as sm,
            tc.tile_pool(name="outp", bufs=2) as outp,
            tc.tile_pool(name="ps_sc", bufs=2, space="PSUM") as ps_sc,
            tc.tile_pool(name="ps_pv", bufs=1, space="PSUM") as ps_pv,
            tc.tile_pool(name="ps_p4", bufs=1, space="PSUM") as ps_p4,
        ):
            # ---- persistent SBUF tensors ----
            wo = big.tile([128, CI, QUERY_DIM], BF, tag="wo", name="wo")
            bo_bc = big.tile([128, QUERY_DIM], F32, tag="bo", name="bo")
            qT8 = big.tile([128, 2, 2, N_DEV], F8, tag="qT8", name="qT8")
            kT8 = big.tile([128, 2, 2, m_pad], F8, tag="kT8", name="kT8")
            v2 = big.tile([128, T, H, 65], BF, tag="v2", name="v2")
            onat = [
                big.tile([128, CI, H, 64], BF, tag=f"onat{qb}", name=f"onat{qb}")
                for qb in range(NQB)
            ]
            oT = [
                big.tile([128, CI, QB], BF, tag=f"oT{qb}", name=f"oT{qb}")
                for qb in range(NQB)
            ]

            wq = big.tile([128, CQ, INNER], BF, tag="wq", name="wq")
            wk = big.tile([128, CC, INNER], BF, tag="wk", name="wk")
            wv = big.tile([128, CC, INNER], BF, tag="wv", name="wv")
            xT = big.tile([128, CQ, N_DEV], BF, tag="xT", name="xT")

            # DMA issue order matters: the sim's DMA-engine pool serializes
            # transfers, so put the startup critical path (wq, xT-half-0 for
            # Q proj qf0; wk, ctxT0 for K proj block 0) ahead of everything.
            nc.gpsimd.dma_start(
                out=wq[:], in_=wq_d[:].rearrange("(o p) f -> p o f", p=128)
            )
            # x^T straight from DRAM via the DMA crossbar
            nc.sync.dma_start_transpose(out=xT[:], in_=xs_d[:])
            nc.gpsimd.dma_start(
                out=wk[:], in_=wk_d[:].rearrange("(o p) f -> p o f", p=128)
            )

            # ---- prologue pieces ----
            def emit_q_proj_qf(qf):
                for dc in range(CI):
                    psq = ps_pv.tile([128, 512], F32, tag="pv", name="psq")
                    for c in range(CQ):
                        nc.tensor.matmul(
                            psq[:],
                            wq[:, c, dc * 128 : (dc + 1) * 128],
                            xT[:, c, qf * 512 : (qf + 1) * 512],
                            start=(c == 0),
                            stop=(c == CQ - 1),
                        )
                    nc.vector.tensor_copy(
                        qT8[:, dc // 2, dc % 2, qf * 512 : (qf + 1) * 512], psq[:]
                    )

            def emit_ctx_block(bi):
                base, bw = MBLK[bi]
                ctxT = ctxt.tile([128, CC, 512], BF, tag="ctxT", name="ctxT")
                nc.sync.dma_start_transpose(
                    out=ctxT[:, :, 0:bw], in_=ctx_d[base : base + bw, :]
                )
                return ctxT

            def emit_k_block(bi, ctxT):
                base, bw = MBLK[bi]
                for dc in range(CI):
                    psk = ps_pv.tile([128, 512], F32, tag="pv", name="psk")
                    for c in range(CC):
                        nc.tensor.matmul(
                            psk[:, :bw],
                            wk[:, c, dc * 128 : (dc + 1) * 128],
                            ctxT[:, c, :bw],
                            start=(c == 0),
                            stop=(c == CC - 1),
                        )
                    nc.vector.tensor_copy(
                        kT8[:, dc // 2, dc % 2, base : base + bw], psk[:, :bw]
                    )

            def emit_v_block(bi, ctxT):
                base, bw = MBLK[bi]
                for ktl in range(bw // 128):
                    t = base // 128 + ktl
                    psv = ps_pv.tile([128, 512], F32, tag="pv", name="psv")
                    for c in range(CC):
                        nc.tensor.matmul(
                            psv[:],
                            ctxT[:, c, ktl * 128 : (ktl + 1) * 128],
                            wv[:, c, :],
                            start=(c == 0),
                            stop=(c == CC - 1),
                        )
                    nc.vector.tensor_copy(
                        v2[:, t, :, 0:64],
                        psv[:].rearrange("p (h d) -> p h d", d=64),
                    )

            # ---- attention unit pieces ----
            def unit_scores_group(qb, h, g0, gn, pb, tays=None):
                hi, b4 = h // 4, h % 4
                p0 = 32 * b4
                sc = ps_sc.tile([128, 1536], F32, tag="sc", name="sc")
                for j in range(gn):
                    t = g0 + j
                    nc.tensor.matmul(
                        sc[:, j * 512 : (j + 1) * 512],
                        kT8[p0 : p0 + 32, hi, :, t * 128 : (t + 1) * 128],
                        qT8[p0 : p0 + 32, hi, :, qb * QB : (qb + 1) * QB],
                        start=True,
                        stop=True,
                        perf_mode=DRM,
                        tile_position=(p0, 0),
                    )
                scv = sc[:, 0 : gn * 512].rearrange("p (g q) -> p g q", q=512)
                if tays is None:
                    nc.scalar.activation(pb[:, g0 : g0 + gn, :], scv, AF.Exp, scale=SCALE)
                else:
                    # offload pass 1 on Pool: t = ALPHA*s + BETA (fp16)
                    ta, _ = tays
                    nc.gpsimd.tensor_scalar(
                        ta[:, g0 : g0 + gn, :],
                        scv,
                        ALPHA,
                        BETA,
                        mybir.AluOpType.mult,
                        mybir.AluOpType.add,
                    )

            def unit_exp_offload_squarings(pb, tays):
                """(t^2+0.5)^16 in two half-unit passes on DVE."""
                ta, tb = tays
                for h0, h1 in HALF:
                    n = h1 - h0
                    a = ta[:, h0:h1, :]
                    b = tb[:, 0:n, :]
                    nc.vector.tensor_mul(b, a, a)  # v = t^2
                    nc.vector.tensor_scalar(
                        a, b, 1.0, 0.5, mybir.AluOpType.mult, mybir.AluOpType.add
                    )  # u = v + 0.5  (= w since w = t^2+0.5... shifted)
                    nc.vector.tensor_mul(b, a, a)  # w^2
                    nc.vector.tensor_mul(a, b, b)  # w^4
                    nc.vector.tensor_mul(b, a, a)  # w^8
                    nc.vector.tensor_mul(pb[:, h0:h1, :], b, b)  # w^16 -> bf16

            def unit_pv_chain(qb, h, c, pb, pv4):
                # all 4 chains of a unit share one PSUM bank at col c*65
                for t in range(T):
                    nc.tensor.matmul(
                        pv4[:, c * 65 : c * 65 + 65],
                        pb[:, t, c * 128 : (c + 1) * 128],
                        v2[:, t, h, :],
                        start=(t == 0),
                        stop=(t == T - 1),
                    )

            def unit_normalize(qb, h, pv4):
                pv4v = pv4[:, 0:260].rearrange("p (c e) -> p c e", e=65)
                linv = sm.tile([128, 4, 1], F32, tag="linv", name="linv")
                nc.vector.reciprocal(linv[:], pv4v[:, :, 64:65])
                nc.vector.tensor_mul(
                    onat[qb][:, :, h, :],
                    pv4v[:, :, 0:64],
                    linv[:].to_broadcast([128, 4, 64]),
                )

            def emit_oT_pair(qb, hp):
                # transpose one head-pair's slice of onat as soon as both
                # heads' normalize is done, instead of all 8 heads at the end
                for qtl in range(CI):
                    nc.sync.dma_start_transpose(
                        out=oT[qb][:, hp, qtl * 128 : (qtl + 1) * 128],
                        in_=onat[qb][:, qtl, 2 * hp : 2 * hp + 2, :],
                    )

            def out_proj_chain(qb, qtl):
                pso = ps_pv.tile([128, 512], F32, tag="pv", name="pso")
                for c in range(CI):
                    nc.tensor.matmul(
                        pso[:],
                        oT[qb][:, c, qtl * 128 : (qtl + 1) * 128],
                        wo[:, c, :],
                        start=(c == 0),
                        stop=(c == CI - 1),
                    )
                ot = outp.tile([128, QUERY_DIM], F32, tag="ot", name="ot")
                nc.vector.tensor_add(ot[:], pso[:], bo_bc[:])
                qt = qb * (QB // 128) + qtl
                nc.sync.dma_start(out=out_d[qt * 128 : (qt + 1) * 128, :], in_=ot[:])

            # ---- emission schedule (software pipeline) ----
            UNITS = [(qb, h) for qb in range(NQB) for h in range(H)]
            NU = len(UNITS)
            PRO = PBUFS  # units whose scores interleave into the prologue
            pbs = {}
            tayss = {}
            done_g = {}  # unit -> next un-emitted score-group index

            def new_unit_bufs(ui):
                pbs[ui] = pbp.tile([128, T, 512], BF, tag="pb", name="pb")
                if ui in dve_units:
                    tayss[ui] = (
                        tay.tile([128, T, 512], FP16, tag="ta", name="ta"),
                        tay.tile([128, TH, 512], FP16, tag="tb", name="tb"),
                    )
                else:
                    tayss[ui] = None

            for ui in range(PRO):
                new_unit_bufs(ui)
                done_g[ui] = 0

            # ctxT0 transpose queued right behind wk, before the cold-path DMAs
            ctxT0 = emit_ctx_block(0)
            nc.gpsimd.dma_start(
                out=wv[:], in_=wv_d[:].rearrange("(o p) f -> p o f", p=128)
            )
            nc.gpsimd.dma_start(
                out=wo[:], in_=wo_d[:].rearrange("(o p) f -> p o f", p=128)
            )
            nc.sync.dma_start(
                out=bo_bc[:],
                in_=bass.AP(tensor=bo_d, offset=0, ap=[[0, 128], [1, QUERY_DIM]]),
            )
            # valid column of v2 straight from DRAM (one DMA per head):
            # v2[p, t, h, 64] = valid[t*128 + p]
            for h in range(H):
                nc.sync.dma_start(
                    out=v2[:, :, h, 64:65],
                    in_=bass.AP(tensor=val_d, offset=0, ap=[[1, 128], [128, T]]),
                )

            emit_q_proj_qf(0)
            emit_q_proj_qf(1)

            # prologue blocks with the first PRO units' score groups greedy
            deferred_v = []
            tiles_ready = 0
            for bi in range(len(MBLK)):
                ctxT = ctxT0 if bi == 0 else emit_ctx_block(bi)
                emit_k_block(bi, ctxT)
                for ui in range(PRO):
                    qb, h = UNITS[ui]
                    tr = tiles_ready + MBLK[bi][1] // 128
                    while (
                        done_g[ui] < len(GROUPS)
                        and GROUPS[done_g[ui]][0] + GROUPS[done_g[ui]][1] <= tr
                    ):
                        g0, gn = GROUPS[done_g[ui]]
                        unit_scores_group(qb, h, g0, gn, pbs[ui], tayss[ui])
                        done_g[ui] += 1
                if bi == 0:
                    emit_v_block(bi, ctxT)
                else:
                    deferred_v.append(bi)
                tiles_ready += MBLK[bi][1] // 128

            for ui in range(PRO):
                if tayss[ui] is not None:
                    unit_exp_offload_squarings(pbs[ui], tayss[ui])

            # ---- steady state: score emission paced against a global filler
            # queue of PV chains / normalizes / out-proj work. Pacing target:
            # by the end of unit fu's score groups the filler has emitted all
            # of unit fu's own PV chains, so PV trails scores by < 1 unit and
            # nothing piles up after the last exp.
            pv4s = {}

            def mk_chain(u, qb, h, c):
                def f():
                    if c == 0:
                        pv4s[u] = ps_p4.tile([128, 512], F32, tag="pv4", name="pv4")
                    unit_pv_chain(qb, h, c, pbs[u], pv4s[u])
                    if c == 3:
                        unit_normalize(qb, h, pv4s[u])
                        if h % 2 == 1:
                            emit_oT_pair(qb, h // 2)
                return f

            flat_fill = []
            # deferred V blocks lead the filler queue (PV chains need v2)
            for bi in list(deferred_v):
                flat_fill.append(
                    lambda bi=bi: emit_v_block(bi, emit_ctx_block(bi))
                )
            chain_end = {}
            for u in range(NU):
                qb, h = UNITS[u]
                for c in range(4):
                    flat_fill.append(mk_chain(u, qb, h, c))
                chain_end[u] = len(flat_fill)
                if h == H - 1:
                    for qtl in range(CI):
                        flat_fill.append(
                            lambda qb=qb, qtl=qtl: out_proj_chain(qb, qtl)
                        )

            fi = [0]

            def pump_to(target):
                while fi[0] < min(target, len(flat_fill)):
                    flat_fill[fi[0]]()
                    fi[0] += 1

            NG = len(GROUPS)
            NFILL = len(flat_fill)
            NSTEADY = NU - PRO
            for fu in range(PRO, NU):
                fqb, fh = UNITS[fu]
                # make sure the pb ring slot this unit reuses is fully retired
                pump_to(chain_end[fu - PBUFS] if fu >= PBUFS else 0)
                new_unit_bufs(fu)
                for gi, (g0, gn) in enumerate(GROUPS):
                    unit_scores_group(fqb, fh, g0, gn, pbs[fu], tayss[fu])
                    # Tile deps are emission-ordered: a PV chain of unit u may
                    # only be emitted once ALL of u's exps are emitted, i.e.
                    # u <= fu-1 while unit fu's groups are in flight.
                    frac = (fu - PRO + (gi + 1) / NG) / (NSTEADY + 1)
                    pump_to(min(chain_end[fu - 1], int(frac * NFILL + 0.5)))
                if tayss[fu] is not None:
                    unit_exp_offload_squarings(pbs[fu], tayss[fu])
            pump_to(NFILL)
            if dbg:
                nc.sync.dma_start(out=dq_d[:], in_=qT8[:])
                nc.sync.dma_start(out=dk_d[:], in_=kT8[:])
                nc.sync.dma_start(out=dv_d[:], in_=v2[:])
                nc.sync.dma_start(out=don_d[:], in_=onat[0][:])
                nc.sync.dma_start(out=dot_d[:], in_=oT[0][:])
                nc.sync.dma_start(out=don1_d[:], in_=onat[1][:])
                nc.sync.dma_start(out=dot1_d[:], in_=oT[1][:])
                nc.sync.dma_start(out=dpb_d[:], in_=pbs[15][:])

    nc.compile()
    return nc


def kernel(x, context_tensor, mask, Wq, Wk, Wv, Wo, bo):
    import ml_dtypes
    from concourse.bass_utils import run_bass_kernel_spmd

    BFnp = ml_dtypes.bfloat16
    x = np.asarray(x, dtype=np.float32)
    context_tensor = np.asarray(context_tensor, dtype=np.float32)
    mask = np.asarray(mask)
    perm = _perm()
    Wq = np.asarray(Wq, dtype=np.float32)[:, perm].astype(BFnp)
    Wk = np.asarray(Wk, dtype=np.float32)[:, perm].astype(BFnp)
    Wv = np.asarray(Wv, dtype=np.float32).astype(BFnp)
    Wo = np.asarray(Wo, dtype=np.float32).astype(BFnp)
    bo = np.ascontiguousarray(np.asarray(bo, dtype=np.float32))

    # host-side context compaction using the mask
    meffs = [int(mask[b].sum()) for b in range(B)]
    m_eff = max(max(meffs), 1)
    m_pad = max(((m_eff + 127) // 128) * 128, M_PAD_MIN)
    ctx_c = np.zeros((B, m_pad, CONTEXT_DIM), dtype=BFnp)
    val = np.zeros((B, m_pad), dtype=BFnp)
    for b in range(B):
        idx = np.flatnonzero(mask[b])
        ctx_c[b, : len(idx)] = context_tensor[b, idx].astype(BFnp)
        val[b, : len(idx)] = 1.0
    xb = x.astype(BFnp)

    if m_pad not in _compiled:
        _compiled[m_pad] = _build(m_pad)
    nc = _compiled[m_pad]

    rows_per_core = N // (NCORES // B)  # 1024
    in_maps = []
    for d in range(NCORES):
        b = d // (NCORES // B)
        r0 = (d % (NCORES // B)) * rows_per_core
        in_maps.append(
            {
                "xs": xb[b, r0 : r0 + rows_per_core],
                "ctx": ctx_c[b],
                "valid": val[b],
                "Wq": Wq,
                "Wk": Wk,
                "Wv": Wv,
                "Wo": Wo,
                "bo": bo,
            }
        )

    res = run_bass_kernel_spmd(nc, in_maps, list(range(NCORES)))
    out = np.empty((B, N, QUERY_DIM), dtype=np.float32)
    for d in range(NCORES):
        b = d // (NCORES // B)
        r0 = (d % (NCORES // B)) * rows_per_core
        out[b, r0 : r0 + rows_per_core] = res.results[d]["out"]
    return out


# revision 39
# speedup vs baseline: 1.1148x; 1.1148x over previous
"""Cross-attention Bass kernel for 8 trn2 NeuronCores — v3.

Sharding: core d handles batch b = d//4, query rows [(d%4)*1024, ...+1024),
all 8 heads (no collectives). Context compacted on host via mask, padded to
m_pad = ceil(max_meff/128)*128 (seed-0 inputs: 2056 -> 2176, 17 k-tiles).

v3 strategy (vs v2 baseline):
- Scores on the PE in fp8e4m3 with MatmulPerfMode.DoubleRow (0.5 cyc/row in
  the cost model vs 1.0 bf16): contraction D=64 packed as [32 partitions, 2]
  pairs. Wq/Wk columns are permuted ON THE HOST so the Q/K projection PSUM
  drains land in the DoubleRow-paired layout with partition-identity casts
  (no repartition pass): column (dc*128+p) holds head h=4*(dc//2)+(p//32),
  dim d=32*(dc%2)+(p%32). Scores stationary = kT8[32b:32b+32, hi, :, mtile],
  moving = qT8[32b:32b+32, hi, :, qblock].
- softmax scale (1/8) folded into ACT exp via activation(scale=...), so q/k
  keep natural magnitude in fp8 (no subnormal loss).
- PV unchanged (bf16, natural orientation, 65th valid column accumulates the
  denominator).
- All 4 PV chains of a unit accumulate into ONE PSUM bank (4x65 f32 <= 512);
  normalize is ONE reciprocal [128,4,1] + ONE broadcast-mul [128,4,64]
  directly from PSUM -> onat (v2 did per-chain recip+mul drains on DVE).
- valid column DMA'd straight from DRAM into v2 (one tiny DMA per head);
  oT transposes emitted per head-pair as soon as both normalizes land.
- Emission is a software pipeline: PE warmup bursts hold the p-state ramp,
  prologue weaves K-chains/V-tiles/Q-chunks between score groups, steady
  state paces a global filler queue (PV/normalize/out-proj) against score
  emission with a 2-items-per-group cap; fillers of unit u emit only after
  ALL of u's exps (Tile deps are emission-ordered).
- Optional exp offload (dve_units): P = (t^2+0.5)^16 with
  t = s*SCALE/(16*sqrt2) + 1/sqrt2 == (1+a+a^2/2)^16, a = s*SCALE/16
  (rel err ~0.1% at |s*SCALE|=1.3). Pass 1 on Pool per score group (PSUM f32
  -> fp16), squarings on DVE in fp16 half-unit buffers, final mul writes pb
  bf16. Off by default; enabled when ACT is the critical engine.
"""
import numpy as np

B, N, M = 2, 4096, 4096
QUERY_DIM, CONTEXT_DIM = 512, 768
H, D = 8, 64
INNER = H * D  # 512
NCORES = 8
N_DEV = (B * N) // NCORES  # 1024 query rows per core
QB = 512
NQB = N_DEV // QB  # 2
SCALE = float(D) ** -0.5  # 0.125
M_PAD_MIN = 128

# exp offload constants: t = ALPHA*s_raw + BETA, P = (t*t+0.5)^16
ALPHA = SCALE / (16.0 * np.sqrt(2.0))
BETA = float(1.0 / np.sqrt(2.0))

DVE_UNITS = ()  # unit indices whose exp runs on Pool+DVE instead of ACT

_compiled = {}


def _perm():
    """Column permutation for Wq/Wk making projection drains land in the
    DoubleRow-paired fp8 layout."""
    perm = np.empty(INNER, dtype=np.int64)
    for dc in range(4):
        hi, i = dc // 2, dc % 2
        for p in range(128):
            b4, p5 = p // 32, p % 32
            h = 4 * hi + b4
            d = 32 * i + p5
            perm[dc * 128 + p] = h * 64 + d
    return perm


def _build(m_pad, dve_units=DVE_UNITS):
    from concourse import bacc
    import concourse.bass as bass
    import concourse.mybir as mybir
    import concourse.tile as tile

    F32 = mybir.dt.float32
    BF = mybir.dt.bfloat16
    FP16 = mybir.dt.float16
    F8 = mybir.dt.float8e4
    AF = mybir.ActivationFunctionType
    DRM = mybir.MatmulPerfMode.DoubleRow

    T = m_pad // 128  # k-tiles
    PBUFS = 5 if T <= 17 else (4 if T <= 19 else 3)
    MBLK = [(0, min(256, m_pad))] + [
        (s, min(512, m_pad - s)) for s in range(256, m_pad, 512)
    ]
    SC_G = 3
    GROUPS = [(0, min(2, T))] + [(g, min(SC_G, T - g)) for g in range(2, T, SC_G)]
    # tile ranges for the two half-unit exp-offload squaring passes
    HALF = [(0, (T + 1) // 2), ((T + 1) // 2, T)]
    TH = max(h1 - h0 for h0, h1 in HALF)

    WARMUP_MM = getattr(_build, "warmup_mm", 20)
    WARMUP2_MM = getattr(_build, "warmup2_mm", 24)
    CQ = QUERY_DIM // 128  # 4
    CC = CONTEXT_DIM // 128  # 6
    CI = INNER // 128  # 4

    nc = bacc.Bacc()
    xs_d = nc.declare_dram_parameter("xs", [N_DEV, QUERY_DIM], BF, isOutput=False)
    ctx_d = nc.declare_dram_parameter("ctx", [m_pad, CONTEXT_DIM], BF, isOutput=False)
    val_d = nc.declare_dram_parameter("valid", [m_pad], BF, isOutput=False)
    wq_d = nc.declare_dram_parameter("Wq", [QUERY_DIM, INNER], BF, isOutput=False)
    wk_d = nc.declare_dram_parameter("Wk", [CONTEXT_DIM, INNER], BF, isOutput=False)
    wv_d = nc.declare_dram_parameter("Wv", [CONTEXT_DIM, INNER], BF, isOutput=False)
    wo_d = nc.declare_dram_parameter("Wo", [INNER, QUERY_DIM], BF, isOutput=False)
    bo_d = nc.declare_dram_parameter("bo", [QUERY_DIM], F32, isOutput=False)
    out_d = nc.declare_dram_parameter("out", [N_DEV, QUERY_DIM], F32, isOutput=True)
    dbg = getattr(_build, "debug", False)
    if dbg:
        dq_d = nc.declare_dram_parameter("dbg_qT8", [128, 2, 2, N_DEV], F8, isOutput=True)
        dk_d = nc.declare_dram_parameter("dbg_kT8", [128, 2, 2, m_pad], F8, isOutput=True)
        dv_d = nc.declare_dram_parameter("dbg_v2", [128, T, H, 65], BF, isOutput=True)
        don_d = nc.declare_dram_parameter("dbg_onat0", [128, CI, H, 64], BF, isOutput=True)
        dot_d = nc.declare_dram_parameter("dbg_oT0", [128, CI, QB], BF, isOutput=True)
        don1_d = nc.declare_dram_parameter("dbg_onat1", [128, CI, H, 64], BF, isOutput=True)
        dot1_d = nc.declare_dram_parameter("dbg_oT1", [128, CI, QB], BF, isOutput=True)
        dpb_d = nc.declare_dram_parameter("dbg_pb15", [128, T, 512], BF, isOutput=True)

    with tile.TileContext(nc) as tc:
        with (
            tc.tile_pool(name="big", bufs=1) as big,
            tc.tile_pool(name="ctxt", bufs=5) as ctxt,
            tc.tile_pool(name="pb", bufs=PBUFS) as pbp,
            tc.tile_pool(name="tay", bufs=2) as tay,
            tc.tile_pool(name="sm", bufs=4) as sm,
            tc.tile_pool(name="outp", bufs=2) as outp,
            tc.tile_pool(name="ps_sc", bufs=2, space="PSUM") as ps_sc,
            tc.tile_pool(name="ps_pv", bufs=1, space="PSUM") as ps_pv,
            tc.tile_pool(name="ps_p4", bufs=1, space="PSUM") as ps_p4,
        ):
            # ---- persistent SBUF tensors ----
            wo = big.tile([128, CI, QUERY_DIM], BF, tag="wo", name="wo")
            bo_bc = big.tile([128, QUERY_DIM], F32, tag="bo", name="bo")
            qT8 = big.tile([128, 2, 2, N_DEV], F8, tag="qT8", name="qT8")
            kT8 = big.tile([128, 2, 2, m_pad], F8, tag="kT8", name="kT8")
            v2 = big.tile([128, T, H, 65], BF, tag="v2", name="v2")
            onat = [
                big.tile([128, CI, H, 64], BF, tag=f"onat{qb}", name=f"onat{qb}")
                for qb in range(NQB)
            ]
            oT = [
                big.tile([128, CI, QB], BF, tag=f"oT{qb}", name=f"oT{qb}")
                for qb in range(NQB)
            ]

            wq = big.tile([128, CQ, INNER], BF, tag="wq", name="wq")
            wk = big.tile([128, CC, INNER], BF, tag="wk", name="wk")
            wv = big.tile([128, CC, INNER], BF, tag="wv", name="wv")
            xT = big.tile([128, CQ, N_DEV], BF, tag="xT", name="xT")

            # DMA issue order matters: the sim's DMA-engine pool serializes
            # transfers, so put the startup critical path (wq, xT-half-0 for
            # Q proj qf0; wk, ctxT0 for K proj block 0) ahead of everything.
            nc.gpsimd.dma_start(
                out=wq[:], in_=wq_d[:].rearrange("(o p) f -> p o f", p=128)
            )
            # x^T straight from DRAM via the DMA crossbar; first half
            # unblocks Q proj qf0 early
            nc.sync.dma_start_transpose(out=xT[:, :, 0:512], in_=xs_d[0:512, :])
            nc.gpsimd.dma_start(
                out=wk[:], in_=wk_d[:].rearrange("(o p) f -> p o f", p=128)
            )

            # ---- prologue pieces ----
            def emit_q_proj_qf(qf, chunks=None):
                for dc in chunks if chunks is not None else range(CI):
                    psq = ps_pv.tile([128, 512], F32, tag="pv", name="psq")
                    for c in range(CQ):
                        nc.tensor.matmul(
                            psq[:],
                            wq[:, c, dc * 128 : (dc + 1) * 128],
                            xT[:, c, qf * 512 : (qf + 1) * 512],
                            start=(c == 0),
                            stop=(c == CQ - 1),
                        )
                    nc.vector.tensor_copy(
                        qT8[:, dc // 2, dc % 2, qf * 512 : (qf + 1) * 512], psq[:]
                    )

            def emit_ctx_block(bi):
                base, bw = MBLK[bi]
                ctxT = ctxt.tile([128, CC, 512], BF, tag="ctxT", name="ctxT")
                nc.sync.dma_start_transpose(
                    out=ctxT[:, :, 0:bw], in_=ctx_d[base : base + bw, :]
                )
                return ctxT

            def emit_k_block(bi, ctxT):
                base, bw = MBLK[bi]
                for dc in range(CI):
                    psk = ps_pv.tile([128, 512], F32, tag="pv", name="psk")
                    for c in range(CC):
                        nc.tensor.matmul(
                            psk[:, :bw],
                            wk[:, c, dc * 128 : (dc + 1) * 128],
                            ctxT[:, c, :bw],
                            start=(c == 0),
                            stop=(c == CC - 1),
                        )
                    nc.vector.tensor_copy(
                        kT8[:, dc // 2, dc % 2, base : base + bw], psk[:, :bw]
                    )

            def emit_v_ktile(bi, ktl, ctxT):
                base, bw = MBLK[bi]
                t = base // 128 + ktl
                psv = ps_pv.tile([128, 512], F32, tag="pv", name="psv")
                for c in range(CC):
                    nc.tensor.matmul(
                        psv[:],
                        ctxT[:, c, ktl * 128 : (ktl + 1) * 128],
                        wv[:, c, :],
                        start=(c == 0),
                        stop=(c == CC - 1),
                    )
                nc.vector.tensor_copy(
                    v2[:, t, :, 0:64],
                    psv[:].rearrange("p (h d) -> p h d", d=64),
                )

            def emit_v_block(bi, ctxT):
                base, bw = MBLK[bi]
                for ktl in range(bw // 128):
                    emit_v_ktile(bi, ktl, ctxT)

            # ---- attention unit pieces ----
            def unit_scores_group(qb, h, g0, gn, pb, tays=None):
                hi, b4 = h // 4, h % 4
                p0 = 32 * b4
                sc = ps_sc.tile([128, 1536], F32, tag="sc", name="sc")
                for j in range(gn):
                    t = g0 + j
                    nc.tensor.matmul(
                        sc[:, j * 512 : (j + 1) * 512],
                        kT8[p0 : p0 + 32, hi, :, t * 128 : (t + 1) * 128],
                        qT8[p0 : p0 + 32, hi, :, qb * QB : (qb + 1) * QB],
                        start=True,
                        stop=True,
                        perf_mode=DRM,
                        tile_position=(p0, 0),
                    )
                scv = sc[:, 0 : gn * 512].rearrange("p (g q) -> p g q", q=512)
                if tays is None:
                    nc.scalar.activation(pb[:, g0 : g0 + gn, :], scv, AF.Exp, scale=SCALE)
                else:
                    # offload pass 1 on Pool: t = ALPHA*s + BETA (fp16)
                    ta, _ = tays
                    nc.gpsimd.tensor_scalar(
                        ta[:, g0 : g0 + gn, :],
                        scv,
                        ALPHA,
                        BETA,
                        mybir.AluOpType.mult,
                        mybir.AluOpType.add,
                    )

            def unit_exp_offload_squarings(pb, tays):
                """(t^2+0.5)^16 in two half-unit passes on DVE."""
                ta, tb = tays
                for h0, h1 in HALF:
                    n = h1 - h0
                    a = ta[:, h0:h1, :]
                    b = tb[:, 0:n, :]
                    nc.vector.tensor_mul(b, a, a)  # v = t^2
                    nc.vector.tensor_scalar(
                        a, b, 1.0, 0.5, mybir.AluOpType.mult, mybir.AluOpType.add
                    )  # u = v + 0.5  (= w since w = t^2+0.5... shifted)
                    nc.vector.tensor_mul(b, a, a)  # w^2
                    nc.vector.tensor_mul(a, b, b)  # w^4
                    nc.vector.tensor_mul(b, a, a)  # w^8
                    nc.vector.tensor_mul(pb[:, h0:h1, :], b, b)  # w^16 -> bf16

            def unit_pv_chain(qb, h, c, pb, pv4):
                # all 4 chains of a unit share one PSUM bank at col c*65
                for t in range(T):
                    nc.tensor.matmul(
                        pv4[:, c * 65 : c * 65 + 65],
                        pb[:, t, c * 128 : (c + 1) * 128],
                        v2[:, t, h, :],
                        start=(t == 0),
                        stop=(t == T - 1),
                    )

            def unit_normalize(qb, h, pv4):
                pv4v = pv4[:, 0:260].rearrange("p (c e) -> p c e", e=65)
                linv = sm.tile([128, 4, 1], F32, tag="linv", name="linv")
                nc.vector.reciprocal(linv[:], pv4v[:, :, 64:65])
                nc.vector.tensor_mul(
                    onat[qb][:, :, h, :],
                    pv4v[:, :, 0:64],
                    linv[:].to_broadcast([128, 4, 64]),
                )

            def emit_oT_pair(qb, hp):
                # transpose one head-pair's slice of onat as soon as both
                # heads' normalize is done, instead of all 8 heads at the end
                for qtl in range(CI):
                    nc.sync.dma_start_transpose(
                        out=oT[qb][:, hp, qtl * 128 : (qtl + 1) * 128],
                        in_=onat[qb][:, qtl, 2 * hp : 2 * hp + 2, :],
                    )

            def out_proj_chain(qb, qtl):
                pso = ps_pv.tile([128, 512], F32, tag="pv", name="pso")
                for c in range(CI):
                    nc.tensor.matmul(
                        pso[:],
                        oT[qb][:, c, qtl * 128 : (qtl + 1) * 128],
                        wo[:, c, :],
                        start=(c == 0),
                        stop=(c == CI - 1),
                    )
                ot = outp.tile([128, QUERY_DIM], F32, tag="ot", name="ot")
                nc.vector.tensor_add(ot[:], pso[:], bo_bc[:])
                qt = qb * (QB // 128) + qtl
                nc.sync.dma_start(out=out_d[qt * 128 : (qt + 1) * 128, :], in_=ot[:])

            # ---- emission schedule (software pipeline) ----
            UNITS = [(qb, h) for qb in range(NQB) for h in range(H)]
            NU = len(UNITS)
            PRO = PBUFS  # units whose scores interleave into the prologue
            pbs = {}
            tayss = {}
            done_g = {}  # unit -> next un-emitted score-group index

            def new_unit_bufs(ui):
                pbs[ui] = pbp.tile([128, T, 512], BF, tag="pb", name="pb")
                if ui in dve_units:
                    tayss[ui] = (
                        tay.tile([128, T, 512], FP16, tag="ta", name="ta"),
                        tay.tile([128, TH, 512], FP16, tag="tb", name="tb"),
                    )
                else:
                    tayss[ui] = None

            for ui in range(PRO):
                new_unit_bufs(ui)
                done_g[ui] = 0

            # ctxT0 transpose queued right behind wk, before the cold-path DMAs
            ctxT0 = emit_ctx_block(0)
            ctxT1_pre = emit_ctx_block(1)
            nc.sync.dma_start_transpose(out=xT[:, :, 512:1024], in_=xs_d[512:1024, :])
            nc.gpsimd.dma_start(
                out=wv[:], in_=wv_d[:].rearrange("(o p) f -> p o f", p=128)
            )
            ctxT2_pre = emit_ctx_block(2)
            ctxT3_pre = emit_ctx_block(3)
            ctxT4_pre = emit_ctx_block(4)
            nc.gpsimd.dma_start(
                out=wo[:], in_=wo_d[:].rearrange("(o p) f -> p o f", p=128)
            )
            nc.sync.dma_start(
                out=bo_bc[:],
                in_=bass.AP(tensor=bo_d, offset=0, ap=[[0, 128], [1, QUERY_DIM]]),
            )
            # valid column of v2 straight from DRAM (one DMA per head):
            # v2[p, t, h, 64] = valid[t*128 + p]
            for h in range(H):
                nc.sync.dma_start(
                    out=v2[:, :, h, 64:65],
                    in_=bass.AP(tensor=val_d, offset=0, ap=[[1, 128], [128, T]]),
                )

            # PE warmup: keep the tensor engine continuously busy from t=0
            # so the p-state ramp reaches full clock before Q proj; matmuls on
            # a zeroed tile, results never read.
            wz = big.tile([128, 640], BF, tag="wz", name="wz")
            nc.vector.memset(wz[:], 0.0)
            for _ in range(WARMUP_MM):
                psw = ps_pv.tile([128, 512], F32, tag="pv", name="psw")
                nc.tensor.matmul(
                    psw[:], wz[:, 0:128], wz[:, 128:640], start=True, stop=True
                )

            # Q proj: just the two chunks units h0-3 need, so the first
            # score group (and ACT) starts as early as possible
            emit_q_proj_qf(0, [0, 1])
            # keep the PE clock hot while wk/ctxT0 land
            for _ in range(WARMUP2_MM):
                psw = ps_pv.tile([128, 512], F32, tag="pv", name="psw")
                nc.tensor.matmul(
                    psw[:], wz[:, 0:128], wz[:, 128:640], start=True, stop=True
                )
            q_hi1 = [False]

            # ---- prologue as a woven stream: block-0 K inline, then score
            # groups of the PRO units alternating with deferred PE work
            # (remaining Q chunks, V0, Q qf1, and the NEXT blocks' K chains)
            # so ACT never waits a whole K-block.
            emit_k_block(0, ctxT0)
            tiles_emitted = [MBLK[0][1] // 128]
            groups_pending = []

            def unlock():
                tr = tiles_emitted[0]
                for ui in range(PRO):
                    qb, h = UNITS[ui]
                    if h >= 4 and not q_hi1[0]:
                        continue
                    while (
                        done_g[ui] < len(GROUPS)
                        and GROUPS[done_g[ui]][0] + GROUPS[done_g[ui]][1] <= tr
                    ):
                        g0, gn = GROUPS[done_g[ui]]
                        groups_pending.append((ui, g0, gn))
                        done_g[ui] += 1

            unlock()

            pro_w = []

            def _qdc23():
                emit_q_proj_qf(0, [2, 3])
                q_hi1[0] = True
                unlock()

            pro_w.append(_qdc23)

            pro_ctx = {0: ctxT0, 1: ctxT1_pre, 2: ctxT2_pre, 3: ctxT3_pre,
                       4: ctxT4_pre}

            def mk_kchain(b, dc):
                def f():
                    if dc == 0 and b not in pro_ctx:
                        pro_ctx[b] = emit_ctx_block(b)
                    base, bw = MBLK[b]
                    psk = ps_pv.tile([128, 512], F32, tag="pv", name="psk")
                    for c in range(CC):
                        nc.tensor.matmul(
                            psk[:, :bw],
                            wk[:, c, dc * 128 : (dc + 1) * 128],
                            pro_ctx[b][:, c, :bw],
                            start=(c == 0),
                            stop=(c == CC - 1),
                        )
                    nc.vector.tensor_copy(
                        kT8[:, dc // 2, dc % 2, base : base + bw], psk[:, :bw]
                    )
                    if dc == CI - 1:
                        tiles_emitted[0] += bw // 128
                        unlock()
                return f

            deferred_v = []
            for b in range(1, len(MBLK)):
                for dc in range(CI):
                    pro_w.append(mk_kchain(b, dc))
                deferred_v.append(b)
                if b == 1:
                    # V0 and Q qf1 wait on the cold DMAs (wv, xT half 2);
                    # schedule them after block 1's K chains so they don't
                    # head-of-line-block the score stream
                    pro_w.append(lambda: emit_v_block(0, ctxT0))
                    for dc in range(CI):
                        pro_w.append(lambda dc=dc: emit_q_proj_qf(1, [dc]))

            wi = [0]
            while groups_pending or wi[0] < len(pro_w):
                if groups_pending:
                    ui, g0, gn = groups_pending.pop(0)
                    qb, h = UNITS[ui]
                    unit_scores_group(qb, h, g0, gn, pbs[ui], tayss[ui])
                if wi[0] < len(pro_w):
                    pro_w[wi[0]]()
                    wi[0] += 1

            for ui in range(PRO):
                if tayss[ui] is not None:
                    unit_exp_offload_squarings(pbs[ui], tayss[ui])

            # ---- steady state: score emission paced against a global filler
            # queue of PV chains / normalizes / out-proj work. Pacing target:
            # by the end of unit fu's score groups the filler has emitted all
            # of unit fu's own PV chains, so PV trails scores by < 1 unit and
            # nothing piles up after the last exp.
            pv4s = {}

            def mk_chain(u, qb, h, c):
                def f():
                    if c == 0:
                        pv4s[u] = ps_p4.tile([128, 512], F32, tag="pv4", name="pv4")
                    unit_pv_chain(qb, h, c, pbs[u], pv4s[u])
                    if c == 3:
                        unit_normalize(qb, h, pv4s[u])
                        if h % 2 == 1:
                            emit_oT_pair(qb, h // 2)
                return f

            flat_fill = []
            # deferred V blocks lead the filler queue (PV chains need v2);
            # per k-tile granularity so the pacing pump stays fine-grained
            for bi in list(deferred_v):
                base, bw = MBLK[bi]
                for ktl in range(bw // 128):
                    flat_fill.append(
                        lambda bi=bi, ktl=ktl: emit_v_ktile(bi, ktl, pro_ctx[bi])
                    )
            chain_end = {}
            for u in range(NU):
                qb, h = UNITS[u]
                for c in range(4):
                    flat_fill.append(mk_chain(u, qb, h, c))
                chain_end[u] = len(flat_fill)
                if h == H - 1:
                    for qtl in range(CI):
                        flat_fill.append(
                            lambda qb=qb, qtl=qtl: out_proj_chain(qb, qtl)
                        )

            fi = [0]

            def pump_to(target):
                while fi[0] < min(target, len(flat_fill)):
                    flat_fill[fi[0]]()
                    fi[0] += 1

            NG = len(GROUPS)
            NFILL = len(flat_fill)
            NSTEADY = NU - PRO
            for fu in range(PRO, NU):
                fqb, fh = UNITS[fu]
                # make sure the pb ring slot this unit reuses is fully retired
                pump_to(chain_end[fu - PBUFS] if fu >= PBUFS else 0)
                new_unit_bufs(fu)
                for gi, (g0, gn) in enumerate(GROUPS):
                    unit_scores_group(fqb, fh, g0, gn, pbs[fu], tayss[fu])
                    # Tile deps are emission-ordered: a PV chain of unit u may
                    # only be emitted once ALL of u's exps are emitted, i.e.
                    # u <= fu-1 while unit fu's groups are in flight.
                    frac = (fu - PRO + (gi + 1) / NG) / (NSTEADY + 1)
                    pump_to(min(chain_end[fu - 1], int(frac * NFILL + 0.5), fi[0] + 2))
                if tayss[fu] is not None:
                    unit_exp_offload_squarings(pbs[fu], tayss[fu])
            pump_to(NFILL)
            if dbg:
                nc.sync.dma_start(out=dq_d[:], in_=qT8[:])
                nc.sync.dma_start(out=dk_d[:], in_=kT8[:])
                nc.sync.dma_start(out=dv_d[:], in_=v2[:])
                nc.sync.dma_start(out=don_d[:], in_=onat[0][:])
                nc.sync.dma_start(out=dot_d[:], in_=oT[0][:])
                nc.sync.dma_start(out=don1_d[:], in_=onat[1][:])
                nc.sync.dma_start(out=dot1_d[:], in_=oT[1][:])
                nc.sync.dma_start(out=dpb_d[:], in_=pbs[15][:])

    nc.compile()
    return nc


def kernel(x, context_tensor, mask, Wq, Wk, Wv, Wo, bo):
    import ml_dtypes
    from concourse.bass_utils import run_bass_kernel_spmd

    BFnp = ml_dtypes.bfloat16
    x = np.asarray(x, dtype=np.float32)
    context_tensor = np.asarray(context_tensor, dtype=np.float32)
    mask = np.asarray(mask)
    perm = _perm()
    Wq = np.asarray(Wq, dtype=np.float32)[:, perm].astype(BFnp)
    Wk = np.asarray(Wk, dtype=np.float32)[:, perm].astype(BFnp)
    Wv = np.asarray(Wv, dtype=np.float32).astype(BFnp)
    Wo = np.asarray(Wo, dtype=np.float32).astype(BFnp)
    bo = np.ascontiguousarray(np.asarray(bo, dtype=np.float32))

    # host-side context compaction using the mask
    meffs = [int(mask[b].sum()) for b in range(B)]
    m_eff = max(max(meffs), 1)
    m_pad = max(((m_eff + 127) // 128) * 128, M_PAD_MIN)
    ctx_c = np.zeros((B, m_pad, CONTEXT_DIM), dtype=BFnp)
    val = np.zeros((B, m_pad), dtype=BFnp)
    for b in range(B):
        idx = np.flatnonzero(mask[b])
        ctx_c[b, : len(idx)] = context_tensor[b, idx].astype(BFnp)
        val[b, : len(idx)] = 1.0
    xb = x.astype(BFnp)

    if m_pad not in _compiled:
        _compiled[m_pad] = _build(m_pad)
    nc = _compiled[m_pad]

    rows_per_core = N // (NCORES // B)  # 1024
    in_maps = []
    for d in range(NCORES):
        b = d // (NCORES // B)
        r0 = (d % (NCORES // B)) * rows_per_core
        in_maps.append(
            {
                "xs": xb[b, r0 : r0 + rows_per_core],
                "ctx": ctx_c[b],
                "valid": val[b],
                "Wq": Wq,
                "Wk": Wk,
                "Wv": Wv,
                "Wo": Wo,
                "bo": bo,
            }
        )

    res = run_bass_kernel_spmd(nc, in_maps, list(range(NCORES)))
    out = np.empty((B, N, QUERY_DIM), dtype=np.float32)
    for d in range(NCORES):
        b = d // (NCORES // B)
        r0 = (d % (NCORES // B)) * rows_per_core
        out[b, r0 : r0 + rows_per_core] = res.results[d]["out"]
    return out


# revision 40
# speedup vs baseline: 1.1274x; 1.0113x over previous
"""Cross-attention Bass kernel for 8 trn2 NeuronCores — v3.

Sharding: core d handles batch b = d//4, query rows [(d%4)*1024, ...+1024),
all 8 heads (no collectives). Context compacted on host via mask, padded to
m_pad = ceil(max_meff/128)*128 (seed-0 inputs: 2056 -> 2176, 17 k-tiles).

v3 strategy (vs v2 baseline):
- Scores on the PE in fp8e4m3 with MatmulPerfMode.DoubleRow (0.5 cyc/row in
  the cost model vs 1.0 bf16): contraction D=64 packed as [32 partitions, 2]
  pairs. Wq/Wk columns are permuted ON THE HOST so the Q/K projection PSUM
  drains land in the DoubleRow-paired layout with partition-identity casts
  (no repartition pass): column (dc*128+p) holds head h=4*(dc//2)+(p//32),
  dim d=32*(dc%2)+(p%32). Scores stationary = kT8[32b:32b+32, hi, :, mtile],
  moving = qT8[32b:32b+32, hi, :, qblock].
- softmax scale (1/8) folded into ACT exp via activation(scale=...), so q/k
  keep natural magnitude in fp8 (no subnormal loss).
- PV unchanged (bf16, natural orientation, 65th valid column accumulates the
  denominator).
- All 4 PV chains of a unit accumulate into ONE PSUM bank (4x65 f32 <= 512);
  normalize is ONE reciprocal [128,4,1] + ONE broadcast-mul [128,4,64]
  directly from PSUM -> onat (v2 did per-chain recip+mul drains on DVE).
- valid column DMA'd straight from DRAM into v2 (one tiny DMA per head);
  oT transposes emitted per head-pair as soon as both normalizes land.
- Emission is a software pipeline: PE warmup bursts hold the p-state ramp,
  prologue weaves K-chains/V-tiles/Q-chunks between score groups, steady
  state paces a global filler queue (PV/normalize/out-proj) against score
  emission with a 2-items-per-group cap; fillers of unit u emit only after
  ALL of u's exps (Tile deps are emission-ordered).
- Optional exp offload (dve_units): P = (t^2+0.5)^16 with
  t = s*SCALE/(16*sqrt2) + 1/sqrt2 == (1+a+a^2/2)^16, a = s*SCALE/16
  (rel err ~0.1% at |s*SCALE|=1.3). Pass 1 on Pool per score group (PSUM f32
  -> fp16), squarings on DVE in fp16 half-unit buffers, final mul writes pb
  bf16. Off by default; enabled when ACT is the critical engine.
"""
import numpy as np

B, N, M = 2, 4096, 4096
QUERY_DIM, CONTEXT_DIM = 512, 768
H, D = 8, 64
INNER = H * D  # 512
NCORES = 8
N_DEV = (B * N) // NCORES  # 1024 query rows per core
QB = 512
NQB = N_DEV // QB  # 2
SCALE = float(D) ** -0.5  # 0.125
M_PAD_MIN = 128

# exp offload constants: t = ALPHA*s_raw + BETA, P = (t*t+0.5)^16
ALPHA = SCALE / (16.0 * np.sqrt(2.0))
BETA = float(1.0 / np.sqrt(2.0))

DVE_UNITS = ()  # unit indices whose exp runs on Pool+DVE instead of ACT

_compiled = {}


def _perm():
    """Column permutation for Wq/Wk making projection drains land in the
    DoubleRow-paired fp8 layout."""
    perm = np.empty(INNER, dtype=np.int64)
    for dc in range(4):
        hi, i = dc // 2, dc % 2
        for p in range(128):
            b4, p5 = p // 32, p % 32
            h = 4 * hi + b4
            d = 32 * i + p5
            perm[dc * 128 + p] = h * 64 + d
    return perm


def _build(m_pad, dve_units=DVE_UNITS):
    from concourse import bacc
    import concourse.bass as bass
    import concourse.mybir as mybir
    import concourse.tile as tile

    F32 = mybir.dt.float32
    BF = mybir.dt.bfloat16
    FP16 = mybir.dt.float16
    F8 = mybir.dt.float8e4
    AF = mybir.ActivationFunctionType
    DRM = mybir.MatmulPerfMode.DoubleRow

    T = m_pad // 128  # k-tiles
    PBUFS = 5 if T <= 17 else (4 if T <= 19 else 3)
    MBLK = [(0, min(256, m_pad))] + [
        (s, min(512, m_pad - s)) for s in range(256, m_pad, 512)
    ]
    SC_G = 3
    GROUPS = [(0, min(2, T))] + [(g, min(SC_G, T - g)) for g in range(2, T, SC_G)]
    # tile ranges for the two half-unit exp-offload squaring passes
    HALF = [(0, (T + 1) // 2), ((T + 1) // 2, T)]
    TH = max(h1 - h0 for h0, h1 in HALF)

    WARMUP_MM = getattr(_build, "warmup_mm", 20)
    WARMUP2_MM = getattr(_build, "warmup2_mm", 24)
    CQ = QUERY_DIM // 128  # 4
    CC = CONTEXT_DIM // 128  # 6
    CI = INNER // 128  # 4

    nc = bacc.Bacc()
    xs_d = nc.declare_dram_parameter("xs", [N_DEV, QUERY_DIM], BF, isOutput=False)
    ctx_d = nc.declare_dram_parameter("ctx", [m_pad, CONTEXT_DIM], BF, isOutput=False)
    val_d = nc.declare_dram_parameter("valid", [m_pad], BF, isOutput=False)
    wq_d = nc.declare_dram_parameter("Wq", [QUERY_DIM, INNER], BF, isOutput=False)
    wk_d = nc.declare_dram_parameter("Wk", [CONTEXT_DIM, INNER], BF, isOutput=False)
    wv_d = nc.declare_dram_parameter("Wv", [CONTEXT_DIM, INNER], BF, isOutput=False)
    wo_d = nc.declare_dram_parameter("Wo", [INNER, QUERY_DIM], BF, isOutput=False)
    bo_d = nc.declare_dram_parameter("bo", [QUERY_DIM], F32, isOutput=False)
    out_d = nc.declare_dram_parameter("out", [N_DEV, QUERY_DIM], BF, isOutput=True)
    dbg = getattr(_build, "debug", False)
    if dbg:
        dq_d = nc.declare_dram_parameter("dbg_qT8", [128, 2, 2, N_DEV], F8, isOutput=True)
        dk_d = nc.declare_dram_parameter("dbg_kT8", [128, 2, 2, m_pad], F8, isOutput=True)
        dv_d = nc.declare_dram_parameter("dbg_v2", [128, T, H, 65], BF, isOutput=True)
        don_d = nc.declare_dram_parameter("dbg_onat0", [128, CI, H, 64], BF, isOutput=True)
        dot_d = nc.declare_dram_parameter("dbg_oT0", [128, CI, QB], BF, isOutput=True)
        don1_d = nc.declare_dram_parameter("dbg_onat1", [128, CI, H, 64], BF, isOutput=True)
        dot1_d = nc.declare_dram_parameter("dbg_oT1", [128, CI, QB], BF, isOutput=True)
        dpb_d = nc.declare_dram_parameter("dbg_pb15", [128, T, 512], BF, isOutput=True)

    with tile.TileContext(nc) as tc:
        with (
            tc.tile_pool(name="big", bufs=1) as big,
            tc.tile_pool(name="ctxt", bufs=5) as ctxt,
            tc.tile_pool(name="pb", bufs=PBUFS) as pbp,
            tc.tile_pool(name="tay", bufs=2) as tay,
            tc.tile_pool(name="sm", bufs=4) as sm,
            tc.tile_pool(name="outp", bufs=2) as outp,
            tc.tile_pool(name="ps_sc", bufs=2, space="PSUM") as ps_sc,
            tc.tile_pool(name="ps_pv", bufs=1, space="PSUM") as ps_pv,
            tc.tile_pool(name="ps_p4", bufs=1, space="PSUM") as ps_p4,
        ):
            # ---- persistent SBUF tensors ----
            wo = big.tile([128, CI, QUERY_DIM], BF, tag="wo", name="wo")
            bo_bc = big.tile([128, QUERY_DIM], F32, tag="bo", name="bo")
            qT8 = big.tile([128, 2, 2, N_DEV], F8, tag="qT8", name="qT8")
            kT8 = big.tile([128, 2, 2, m_pad], F8, tag="kT8", name="kT8")
            v2 = big.tile([128, T, H, 65], BF, tag="v2", name="v2")
            onat = [
                big.tile([128, CI, H, 64], BF, tag=f"onat{qb}", name=f"onat{qb}")
                for qb in range(NQB)
            ]
            oT = [
                big.tile([128, CI, QB], BF, tag=f"oT{qb}", name=f"oT{qb}")
                for qb in range(NQB)
            ]

            wq = big.tile([128, CQ, INNER], BF, tag="wq", name="wq")
            wk = big.tile([128, CC, INNER], BF, tag="wk", name="wk")
            wv = big.tile([128, CC, INNER], BF, tag="wv", name="wv")
            xT = big.tile([128, CQ, N_DEV], BF, tag="xT", name="xT")

            # DMA issue order matters: the sim's DMA-engine pool serializes
            # transfers, so put the startup critical path (wq, xT-half-0 for
            # Q proj qf0; wk, ctxT0 for K proj block 0) ahead of everything.
            nc.gpsimd.dma_start(
                out=wq[:], in_=wq_d[:].rearrange("(o p) f -> p o f", p=128)
            )
            # x^T straight from DRAM via the DMA crossbar; first half
            # unblocks Q proj qf0 early
            nc.sync.dma_start_transpose(out=xT[:, :, 0:512], in_=xs_d[0:512, :])
            nc.gpsimd.dma_start(
                out=wk[:], in_=wk_d[:].rearrange("(o p) f -> p o f", p=128)
            )

            # ---- prologue pieces ----
            def emit_q_proj_qf(qf, chunks=None):
                for dc in chunks if chunks is not None else range(CI):
                    psq = ps_pv.tile([128, 512], F32, tag="pv", name="psq")
                    for c in range(CQ):
                        nc.tensor.matmul(
                            psq[:],
                            wq[:, c, dc * 128 : (dc + 1) * 128],
                            xT[:, c, qf * 512 : (qf + 1) * 512],
                            start=(c == 0),
                            stop=(c == CQ - 1),
                        )
                    nc.vector.tensor_copy(
                        qT8[:, dc // 2, dc % 2, qf * 512 : (qf + 1) * 512], psq[:]
                    )

            def emit_ctx_block(bi):
                base, bw = MBLK[bi]
                ctxT = ctxt.tile([128, CC, 512], BF, tag="ctxT", name="ctxT")
                nc.sync.dma_start_transpose(
                    out=ctxT[:, :, 0:bw], in_=ctx_d[base : base + bw, :]
                )
                return ctxT

            def emit_k_block(bi, ctxT):
                base, bw = MBLK[bi]
                for dc in range(CI):
                    psk = ps_pv.tile([128, 512], F32, tag="pv", name="psk")
                    for c in range(CC):
                        nc.tensor.matmul(
                            psk[:, :bw],
                            wk[:, c, dc * 128 : (dc + 1) * 128],
                            ctxT[:, c, :bw],
                            start=(c == 0),
                            stop=(c == CC - 1),
                        )
                    nc.vector.tensor_copy(
                        kT8[:, dc // 2, dc % 2, base : base + bw], psk[:, :bw]
                    )

            def emit_v_ktile(bi, ktl, ctxT):
                base, bw = MBLK[bi]
                t = base // 128 + ktl
                psv = ps_pv.tile([128, 512], F32, tag="pv", name="psv")
                for c in range(CC):
                    nc.tensor.matmul(
                        psv[:],
                        ctxT[:, c, ktl * 128 : (ktl + 1) * 128],
                        wv[:, c, :],
                        start=(c == 0),
                        stop=(c == CC - 1),
                    )
                nc.vector.tensor_copy(
                    v2[:, t, :, 0:64],
                    psv[:].rearrange("p (h d) -> p h d", d=64),
                )

            def emit_v_block(bi, ctxT):
                base, bw = MBLK[bi]
                for ktl in range(bw // 128):
                    emit_v_ktile(bi, ktl, ctxT)

            # ---- attention unit pieces ----
            def unit_scores_group(qb, h, g0, gn, pb, tays=None):
                hi, b4 = h // 4, h % 4
                p0 = 32 * b4
                sc = ps_sc.tile([128, 1536], F32, tag="sc", name="sc")
                for j in range(gn):
                    t = g0 + j
                    nc.tensor.matmul(
                        sc[:, j * 512 : (j + 1) * 512],
                        kT8[p0 : p0 + 32, hi, :, t * 128 : (t + 1) * 128],
                        qT8[p0 : p0 + 32, hi, :, qb * QB : (qb + 1) * QB],
                        start=True,
                        stop=True,
                        perf_mode=DRM,
                        tile_position=(p0, 0),
                    )
                scv = sc[:, 0 : gn * 512].rearrange("p (g q) -> p g q", q=512)
                if tays is None:
                    nc.scalar.activation(pb[:, g0 : g0 + gn, :], scv, AF.Exp, scale=SCALE)
                else:
                    # offload pass 1 on Pool: t = ALPHA*s + BETA (fp16)
                    ta, _ = tays
                    nc.gpsimd.tensor_scalar(
                        ta[:, g0 : g0 + gn, :],
                        scv,
                        ALPHA,
                        BETA,
                        mybir.AluOpType.mult,
                        mybir.AluOpType.add,
                    )

            def unit_exp_offload_squarings(pb, tays):
                """(t^2+0.5)^16 in two half-unit passes on DVE."""
                ta, tb = tays
                for h0, h1 in HALF:
                    n = h1 - h0
                    a = ta[:, h0:h1, :]
                    b = tb[:, 0:n, :]
                    nc.vector.tensor_mul(b, a, a)  # v = t^2
                    nc.vector.tensor_scalar(
                        a, b, 1.0, 0.5, mybir.AluOpType.mult, mybir.AluOpType.add
                    )  # u = v + 0.5  (= w since w = t^2+0.5... shifted)
                    nc.vector.tensor_mul(b, a, a)  # w^2
                    nc.vector.tensor_mul(a, b, b)  # w^4
                    nc.vector.tensor_mul(b, a, a)  # w^8
                    nc.vector.tensor_mul(pb[:, h0:h1, :], b, b)  # w^16 -> bf16

            def unit_pv_chain(qb, h, c, pb, pv4):
                # all 4 chains of a unit share one PSUM bank at col c*65
                for t in range(T):
                    nc.tensor.matmul(
                        pv4[:, c * 65 : c * 65 + 65],
                        pb[:, t, c * 128 : (c + 1) * 128],
                        v2[:, t, h, :],
                        start=(t == 0),
                        stop=(t == T - 1),
                    )

            def unit_normalize(qb, h, pv4):
                pv4v = pv4[:, 0:260].rearrange("p (c e) -> p c e", e=65)
                linv = sm.tile([128, 4, 1], F32, tag="linv", name="linv")
                nc.vector.reciprocal(linv[:], pv4v[:, :, 64:65])
                nc.vector.tensor_mul(
                    onat[qb][:, :, h, :],
                    pv4v[:, :, 0:64],
                    linv[:].to_broadcast([128, 4, 64]),
                )

            def emit_oT_pair(qb, hp):
                # transpose one head-pair's slice of onat as soon as both
                # heads' normalize is done; stage it contiguous on DVE so a
                # SINGLE crossbar transpose covers all 4 q-subtiles (one
                # 625ns HWDGE stage instead of four)
                stgt = sm.tile([128, 4, 2, 64], BF, tag="otst", name="otst")
                nc.vector.tensor_copy(stgt[:], onat[qb][:, :, 2 * hp : 2 * hp + 2, :])
                nc.sync.dma_start_transpose(
                    out=oT[qb][:, hp, :].rearrange("p (t q) -> p t q", q=128),
                    in_=stgt[:],
                )

            def out_proj_chain(qb, qtl):
                pso = ps_pv.tile([128, 512], F32, tag="pv", name="pso")
                for c in range(CI):
                    nc.tensor.matmul(
                        pso[:],
                        oT[qb][:, c, qtl * 128 : (qtl + 1) * 128],
                        wo[:, c, :],
                        start=(c == 0),
                        stop=(c == CI - 1),
                    )
                ot = outp.tile([128, QUERY_DIM], BF, tag="ot", name="ot")
                nc.vector.tensor_add(ot[:], pso[:], bo_bc[:])
                qt = qb * (QB // 128) + qtl
                nc.sync.dma_start(out=out_d[qt * 128 : (qt + 1) * 128, :], in_=ot[:])

            # ---- emission schedule (software pipeline) ----
            UNITS = [(qb, h) for qb in range(NQB) for h in range(H)]
            NU = len(UNITS)
            PRO = PBUFS  # units whose scores interleave into the prologue
            pbs = {}
            tayss = {}
            done_g = {}  # unit -> next un-emitted score-group index

            def new_unit_bufs(ui):
                pbs[ui] = pbp.tile([128, T, 512], BF, tag="pb", name="pb")
                if ui in dve_units:
                    tayss[ui] = (
                        tay.tile([128, T, 512], FP16, tag="ta", name="ta"),
                        tay.tile([128, TH, 512], FP16, tag="tb", name="tb"),
                    )
                else:
                    tayss[ui] = None

            for ui in range(PRO):
                new_unit_bufs(ui)
                done_g[ui] = 0

            # ctxT0 transpose queued right behind wk, before the cold-path DMAs
            ctxT0 = emit_ctx_block(0)
            ctxT1_pre = emit_ctx_block(1)
            nc.sync.dma_start_transpose(out=xT[:, :, 512:1024], in_=xs_d[512:1024, :])
            nc.gpsimd.dma_start(
                out=wv[:], in_=wv_d[:].rearrange("(o p) f -> p o f", p=128)
            )
            ctxT2_pre = emit_ctx_block(2)
            ctxT3_pre = emit_ctx_block(3)
            ctxT4_pre = emit_ctx_block(4)
            nc.gpsimd.dma_start(
                out=wo[:], in_=wo_d[:].rearrange("(o p) f -> p o f", p=128)
            )
            nc.sync.dma_start(
                out=bo_bc[:],
                in_=bass.AP(tensor=bo_d, offset=0, ap=[[0, 128], [1, QUERY_DIM]]),
            )
            # valid column of v2 straight from DRAM (one DMA per head):
            # v2[p, t, h, 64] = valid[t*128 + p]
            for h in range(H):
                nc.sync.dma_start(
                    out=v2[:, :, h, 64:65],
                    in_=bass.AP(tensor=val_d, offset=0, ap=[[1, 128], [128, T]]),
                )

            # PE warmup: keep the tensor engine continuously busy from t=0
            # so the p-state ramp reaches full clock before Q proj; matmuls on
            # a zeroed tile, results never read.
            wz = big.tile([128, 640], BF, tag="wz", name="wz")
            nc.vector.memset(wz[:], 0.0)
            for _ in range(WARMUP_MM):
                psw = ps_pv.tile([128, 512], F32, tag="pv", name="psw")
                nc.tensor.matmul(
                    psw[:], wz[:, 0:128], wz[:, 128:640], start=True, stop=True
                )

            # Q proj: just the two chunks units h0-3 need, so the first
            # score group (and ACT) starts as early as possible
            emit_q_proj_qf(0, [0, 1])
            # keep the PE clock hot while wk/ctxT0 land
            for _ in range(WARMUP2_MM):
                psw = ps_pv.tile([128, 512], F32, tag="pv", name="psw")
                nc.tensor.matmul(
                    psw[:], wz[:, 0:128], wz[:, 128:640], start=True, stop=True
                )
            q_hi1 = [False]

            # ---- prologue as a woven stream: block-0 K inline, then score
            # groups of the PRO units alternating with deferred PE work
            # (remaining Q chunks, V0, Q qf1, and the NEXT blocks' K chains)
            # so ACT never waits a whole K-block.
            emit_k_block(0, ctxT0)
            tiles_emitted = [MBLK[0][1] // 128]
            groups_pending = []

            def unlock():
                tr = tiles_emitted[0]
                for ui in range(PRO):
                    qb, h = UNITS[ui]
                    if h >= 4 and not q_hi1[0]:
                        continue
                    while (
                        done_g[ui] < len(GROUPS)
                        and GROUPS[done_g[ui]][0] + GROUPS[done_g[ui]][1] <= tr
                    ):
                        g0, gn = GROUPS[done_g[ui]]
                        groups_pending.append((ui, g0, gn))
                        done_g[ui] += 1

            unlock()

            pro_w = []

            def _qdc23():
                emit_q_proj_qf(0, [2, 3])
                q_hi1[0] = True
                unlock()

            pro_w.append(_qdc23)

            pro_ctx = {0: ctxT0, 1: ctxT1_pre, 2: ctxT2_pre, 3: ctxT3_pre,
                       4: ctxT4_pre}

            def mk_kchain(b, dc):
                def f():
                    if dc == 0 and b not in pro_ctx:
                        pro_ctx[b] = emit_ctx_block(b)
                    base, bw = MBLK[b]
                    psk = ps_pv.tile([128, 512], F32, tag="pv", name="psk")
                    for c in range(CC):
                        nc.tensor.matmul(
                            psk[:, :bw],
                            wk[:, c, dc * 128 : (dc + 1) * 128],
                            pro_ctx[b][:, c, :bw],
                            start=(c == 0),
                            stop=(c == CC - 1),
                        )
                    nc.vector.tensor_copy(
                        kT8[:, dc // 2, dc % 2, base : base + bw], psk[:, :bw]
                    )
                    if dc == CI - 1:
                        tiles_emitted[0] += bw // 128
                        unlock()
                return f

            deferred_v = []
            for b in range(1, len(MBLK)):
                for dc in range(CI):
                    pro_w.append(mk_kchain(b, dc))
                deferred_v.append(b)
                if b == 1:
                    # V0 and Q qf1 wait on the cold DMAs (wv, xT half 2);
                    # schedule them after block 1's K chains so they don't
                    # head-of-line-block the score stream
                    pro_w.append(lambda: emit_v_block(0, ctxT0))
                    for dc in range(CI):
                        pro_w.append(lambda dc=dc: emit_q_proj_qf(1, [dc]))

            wi = [0]
            while groups_pending or wi[0] < len(pro_w):
                if groups_pending:
                    ui, g0, gn = groups_pending.pop(0)
                    qb, h = UNITS[ui]
                    unit_scores_group(qb, h, g0, gn, pbs[ui], tayss[ui])
                if wi[0] < len(pro_w):
                    pro_w[wi[0]]()
                    wi[0] += 1

            for ui in range(PRO):
                if tayss[ui] is not None:
                    unit_exp_offload_squarings(pbs[ui], tayss[ui])

            # ---- steady state: score emission paced against a global filler
            # queue of PV chains / normalizes / out-proj work. Pacing target:
            # by the end of unit fu's score groups the filler has emitted all
            # of unit fu's own PV chains, so PV trails scores by < 1 unit and
            # nothing piles up after the last exp.
            pv4s = {}

            def mk_chain(u, qb, h, c):
                def f():
                    if c == 0:
                        pv4s[u] = ps_p4.tile([128, 512], F32, tag="pv4", name="pv4")
                    unit_pv_chain(qb, h, c, pbs[u], pv4s[u])
                    if c == 3:
                        unit_normalize(qb, h, pv4s[u])
                        if h % 2 == 1:
                            emit_oT_pair(qb, h // 2)
                return f

            flat_fill = []
            # deferred V blocks lead the filler queue (PV chains need v2);
            # per k-tile granularity so the pacing pump stays fine-grained
            for bi in list(deferred_v):
                base, bw = MBLK[bi]
                for ktl in range(bw // 128):
                    flat_fill.append(
                        lambda bi=bi, ktl=ktl: emit_v_ktile(bi, ktl, pro_ctx[bi])
                    )
            chain_end = {}
            for u in range(NU):
                qb, h = UNITS[u]
                for c in range(4):
                    flat_fill.append(mk_chain(u, qb, h, c))
                chain_end[u] = len(flat_fill)
                if h == H - 1:
                    for qtl in range(CI):
                        flat_fill.append(
                            lambda qb=qb, qtl=qtl: out_proj_chain(qb, qtl)
                        )

            fi = [0]

            def pump_to(target):
                while fi[0] < min(target, len(flat_fill)):
                    flat_fill[fi[0]]()
                    fi[0] += 1

            NG = len(GROUPS)
            NFILL = len(flat_fill)
            NSTEADY = NU - PRO
            for fu in range(PRO, NU):
                fqb, fh = UNITS[fu]
                # make sure the pb ring slot this unit reuses is fully retired
                pump_to(chain_end[fu - PBUFS] if fu >= PBUFS else 0)
                new_unit_bufs(fu)
                for gi, (g0, gn) in enumerate(GROUPS):
                    unit_scores_group(fqb, fh, g0, gn, pbs[fu], tayss[fu])
                    # Tile deps are emission-ordered: a PV chain of unit u may
                    # only be emitted once ALL of u's exps are emitted, i.e.
                    # u <= fu-1 while unit fu's groups are in flight.
                    frac = (fu - PRO + (gi + 1) / NG) / (NSTEADY + 1)
                    pump_to(min(chain_end[fu - 1], int(frac * NFILL + 0.5), fi[0] + 2))
                if tayss[fu] is not None:
                    unit_exp_offload_squarings(pbs[fu], tayss[fu])
            pump_to(NFILL)
            if dbg:
                nc.sync.dma_start(out=dq_d[:], in_=qT8[:])
                nc.sync.dma_start(out=dk_d[:], in_=kT8[:])
                nc.sync.dma_start(out=dv_d[:], in_=v2[:])
                nc.sync.dma_start(out=don_d[:], in_=onat[0][:])
                nc.sync.dma_start(out=dot_d[:], in_=oT[0][:])
                nc.sync.dma_start(out=don1_d[:], in_=onat[1][:])
                nc.sync.dma_start(out=dot1_d[:], in_=oT[1][:])
                nc.sync.dma_start(out=dpb_d[:], in_=pbs[15][:])

    nc.compile()
    return nc


def kernel(x, context_tensor, mask, Wq, Wk, Wv, Wo, bo):
    import ml_dtypes
    from concourse.bass_utils import run_bass_kernel_spmd

    BFnp = ml_dtypes.bfloat16
    x = np.asarray(x, dtype=np.float32)
    context_tensor = np.asarray(context_tensor, dtype=np.float32)
    mask = np.asarray(mask)
    perm = _perm()
    Wq = np.asarray(Wq, dtype=np.float32)[:, perm].astype(BFnp)
    Wk = np.asarray(Wk, dtype=np.float32)[:, perm].astype(BFnp)
    Wv = np.asarray(Wv, dtype=np.float32).astype(BFnp)
    Wo = np.asarray(Wo, dtype=np.float32).astype(BFnp)
    bo = np.ascontiguousarray(np.asarray(bo, dtype=np.float32))

    # host-side context compaction using the mask
    meffs = [int(mask[b].sum()) for b in range(B)]
    m_eff = max(max(meffs), 1)
    m_pad = max(((m_eff + 127) // 128) * 128, M_PAD_MIN)
    ctx_c = np.zeros((B, m_pad, CONTEXT_DIM), dtype=BFnp)
    val = np.zeros((B, m_pad), dtype=BFnp)
    for b in range(B):
        idx = np.flatnonzero(mask[b])
        ctx_c[b, : len(idx)] = context_tensor[b, idx].astype(BFnp)
        val[b, : len(idx)] = 1.0
    xb = x.astype(BFnp)

    if m_pad not in _compiled:
        _compiled[m_pad] = _build(m_pad)
    nc = _compiled[m_pad]

    rows_per_core = N // (NCORES // B)  # 1024
    in_maps = []
    for d in range(NCORES):
        b = d // (NCORES // B)
        r0 = (d % (NCORES // B)) * rows_per_core
        in_maps.append(
            {
                "xs": xb[b, r0 : r0 + rows_per_core],
                "ctx": ctx_c[b],
                "valid": val[b],
                "Wq": Wq,
                "Wk": Wk,
                "Wv": Wv,
                "Wo": Wo,
                "bo": bo,
            }
        )

    res = run_bass_kernel_spmd(nc, in_maps, list(range(NCORES)))
    out = np.empty((B, N, QUERY_DIM), dtype=np.float32)
    for d in range(NCORES):
        b = d // (NCORES // B)
        r0 = (d % (NCORES // B)) * rows_per_core
        out[b, r0 : r0 + rows_per_core] = res.results[d]["out"].astype(np.float32)
    return out


# revision 41
# speedup vs baseline: 1.1282x; 1.0008x over previous
"""Cross-attention Bass kernel for 8 trn2 NeuronCores — v3.

Sharding: core d handles batch b = d//4, query rows [(d%4)*1024, ...+1024),
all 8 heads (no collectives). Context compacted on host via mask, padded to
m_pad = ceil(max_meff/128)*128 (seed-0 inputs: 2056 -> 2176, 17 k-tiles).

v3 strategy (vs v2 baseline):
- Scores on the PE in fp8e4m3 with MatmulPerfMode.DoubleRow (0.5 cyc/row in
  the cost model vs 1.0 bf16): contraction D=64 packed as [32 partitions, 2]
  pairs. Wq/Wk columns are permuted ON THE HOST so the Q/K projection PSUM
  drains land in the DoubleRow-paired layout with partition-identity casts
  (no repartition pass): column (dc*128+p) holds head h=4*(dc//2)+(p//32),
  dim d=32*(dc%2)+(p%32). Scores stationary = kT8[32b:32b+32, hi, :, mtile],
  moving = qT8[32b:32b+32, hi, :, qblock].
- softmax scale (1/8) folded into ACT exp via activation(scale=...), so q/k
  keep natural magnitude in fp8 (no subnormal loss).
- PV unchanged (bf16, natural orientation, 65th valid column accumulates the
  denominator).
- All 4 PV chains of a unit accumulate into ONE PSUM bank (4x65 f32 <= 512);
  normalize is ONE reciprocal [128,4,1] + ONE broadcast-mul [128,4,64]
  directly from PSUM -> onat (v2 did per-chain recip+mul drains on DVE).
- valid column DMA'd straight from DRAM into v2 (one tiny DMA per head);
  oT transposes emitted per head-pair as soon as both normalizes land.
- Emission is a software pipeline: PE warmup bursts hold the p-state ramp,
  prologue weaves K-chains/V-tiles/Q-chunks between score groups, steady
  state paces a global filler queue (PV/normalize/out-proj) against score
  emission with a 2-items-per-group cap; fillers of unit u emit only after
  ALL of u's exps (Tile deps are emission-ordered).
- Optional exp offload (dve_units): P = (t^2+0.5)^16 with
  t = s*SCALE/(16*sqrt2) + 1/sqrt2 == (1+a+a^2/2)^16, a = s*SCALE/16
  (rel err ~0.1% at |s*SCALE|=1.3). Pass 1 on Pool per score group (PSUM f32
  -> fp16), squarings on DVE in fp16 half-unit buffers, final mul writes pb
  bf16. Off by default; enabled when ACT is the critical engine.
"""
import numpy as np

B, N, M = 2, 4096, 4096
QUERY_DIM, CONTEXT_DIM = 512, 768
H, D = 8, 64
INNER = H * D  # 512
NCORES = 8
N_DEV = (B * N) // NCORES  # 1024 query rows per core
QB = 512
NQB = N_DEV // QB  # 2
SCALE = float(D) ** -0.5  # 0.125
M_PAD_MIN = 128

# exp offload constants: t = ALPHA*s_raw + BETA, P = (t*t+0.5)^16
ALPHA = SCALE / (16.0 * np.sqrt(2.0))
BETA = float(1.0 / np.sqrt(2.0))

DVE_UNITS = ()  # unit indices whose exp runs on Pool+DVE instead of ACT

_compiled = {}


def _perm():
    """Column permutation for Wq/Wk making projection drains land in the
    DoubleRow-paired fp8 layout."""
    perm = np.empty(INNER, dtype=np.int64)
    for dc in range(4):
        hi, i = dc // 2, dc % 2
        for p in range(128):
            b4, p5 = p // 32, p % 32
            h = 4 * hi + b4
            d = 32 * i + p5
            perm[dc * 128 + p] = h * 64 + d
    return perm


def _build(m_pad, dve_units=DVE_UNITS):
    from concourse import bacc
    import concourse.bass as bass
    import concourse.mybir as mybir
    import concourse.tile as tile

    F32 = mybir.dt.float32
    BF = mybir.dt.bfloat16
    FP16 = mybir.dt.float16
    F8 = mybir.dt.float8e4
    AF = mybir.ActivationFunctionType
    DRM = mybir.MatmulPerfMode.DoubleRow

    T = m_pad // 128  # k-tiles
    PBUFS = 5 if T <= 17 else (4 if T <= 19 else 3)
    MBLK = [(0, min(256, m_pad))] + [
        (s, min(512, m_pad - s)) for s in range(256, m_pad, 512)
    ]
    SC_G = 3
    GROUPS = [(0, min(2, T))] + [(g, min(SC_G, T - g)) for g in range(2, T, SC_G)]
    # tile ranges for the two half-unit exp-offload squaring passes
    HALF = [(0, (T + 1) // 2), ((T + 1) // 2, T)]
    TH = max(h1 - h0 for h0, h1 in HALF)

    WARMUP_MM = getattr(_build, "warmup_mm", 20)
    WARMUP2_MM = getattr(_build, "warmup2_mm", 24)
    CQ = QUERY_DIM // 128  # 4
    CC = CONTEXT_DIM // 128  # 6
    CI = INNER // 128  # 4

    nc = bacc.Bacc()
    xs_d = nc.declare_dram_parameter("xs", [N_DEV, QUERY_DIM], BF, isOutput=False)
    ctx_d = nc.declare_dram_parameter("ctx", [m_pad, CONTEXT_DIM], BF, isOutput=False)
    val_d = nc.declare_dram_parameter("valid", [m_pad], BF, isOutput=False)
    wq_d = nc.declare_dram_parameter("Wq", [QUERY_DIM, INNER], BF, isOutput=False)
    wk_d = nc.declare_dram_parameter("Wk", [CONTEXT_DIM, INNER], BF, isOutput=False)
    wv_d = nc.declare_dram_parameter("Wv", [CONTEXT_DIM, INNER], BF, isOutput=False)
    wo_d = nc.declare_dram_parameter("Wo", [INNER, QUERY_DIM], BF, isOutput=False)
    bo_d = nc.declare_dram_parameter("bo", [QUERY_DIM], F32, isOutput=False)
    out_d = nc.declare_dram_parameter("out", [N_DEV, QUERY_DIM], BF, isOutput=True)
    dbg = getattr(_build, "debug", False)
    if dbg:
        dq_d = nc.declare_dram_parameter("dbg_qT8", [128, 2, 2, N_DEV], F8, isOutput=True)
        dk_d = nc.declare_dram_parameter("dbg_kT8", [128, 2, 2, m_pad], F8, isOutput=True)
        dv_d = nc.declare_dram_parameter("dbg_v2", [128, T, H, 65], BF, isOutput=True)
        don_d = nc.declare_dram_parameter("dbg_onat0", [128, CI, H, 64], BF, isOutput=True)
        dot_d = nc.declare_dram_parameter("dbg_oT0", [128, CI, QB], BF, isOutput=True)
        don1_d = nc.declare_dram_parameter("dbg_onat1", [128, CI, H, 64], BF, isOutput=True)
        dot1_d = nc.declare_dram_parameter("dbg_oT1", [128, CI, QB], BF, isOutput=True)
        dpb_d = nc.declare_dram_parameter("dbg_pb15", [128, T, 512], BF, isOutput=True)

    with tile.TileContext(nc) as tc:
        with (
            tc.tile_pool(name="big", bufs=1) as big,
            tc.tile_pool(name="ctxt", bufs=5) as ctxt,
            tc.tile_pool(name="pb", bufs=PBUFS) as pbp,
            tc.tile_pool(name="tay", bufs=2) as tay,
            tc.tile_pool(name="sm", bufs=4) as sm,
            tc.tile_pool(name="outp", bufs=2) as outp,
            tc.tile_pool(name="ps_sc", bufs=2, space="PSUM") as ps_sc,
            tc.tile_pool(name="ps_pv", bufs=1, space="PSUM") as ps_pv,
            tc.tile_pool(name="ps_p4", bufs=1, space="PSUM") as ps_p4,
        ):
            # ---- persistent SBUF tensors ----
            wo = big.tile([128, CI, QUERY_DIM], BF, tag="wo", name="wo")
            bo_bc = big.tile([128, QUERY_DIM], F32, tag="bo", name="bo")
            qT8 = big.tile([128, 2, 2, N_DEV], F8, tag="qT8", name="qT8")
            kT8 = big.tile([128, 2, 2, m_pad], F8, tag="kT8", name="kT8")
            v2 = big.tile([128, T, H, 65], BF, tag="v2", name="v2")
            onat = [
                big.tile([128, CI, H, 64], BF, tag=f"onat{qb}", name=f"onat{qb}")
                for qb in range(NQB)
            ]
            oT = [
                big.tile([128, CI, QB], BF, tag=f"oT{qb}", name=f"oT{qb}")
                for qb in range(NQB)
            ]

            wq = big.tile([128, CQ, INNER], BF, tag="wq", name="wq")
            wk = big.tile([128, CC, INNER], BF, tag="wk", name="wk")
            wv = big.tile([128, CC, INNER], BF, tag="wv", name="wv")
            xT = big.tile([128, CQ, N_DEV], BF, tag="xT", name="xT")

            # DMA issue order matters: the sim's DMA-engine pool serializes
            # transfers, so put the startup critical path (wq, xT-half-0 for
            # Q proj qf0; wk, ctxT0 for K proj block 0) ahead of everything.
            nc.gpsimd.dma_start(
                out=wq[:], in_=wq_d[:].rearrange("(o p) f -> p o f", p=128)
            )
            # x^T straight from DRAM via the DMA crossbar; first half
            # unblocks Q proj qf0 early
            nc.sync.dma_start_transpose(out=xT[:, :, 0:512], in_=xs_d[0:512, :])
            nc.gpsimd.dma_start(
                out=wk[:], in_=wk_d[:].rearrange("(o p) f -> p o f", p=128)
            )

            # ---- prologue pieces ----
            def emit_q_proj_qf(qf, chunks=None):
                for dc in chunks if chunks is not None else range(CI):
                    psq = ps_pv.tile([128, 512], F32, tag="pv", name="psq")
                    for c in range(CQ):
                        nc.tensor.matmul(
                            psq[:],
                            wq[:, c, dc * 128 : (dc + 1) * 128],
                            xT[:, c, qf * 512 : (qf + 1) * 512],
                            start=(c == 0),
                            stop=(c == CQ - 1),
                        )
                    nc.vector.tensor_copy(
                        qT8[:, dc // 2, dc % 2, qf * 512 : (qf + 1) * 512], psq[:]
                    )

            def emit_ctx_block(bi):
                base, bw = MBLK[bi]
                ctxT = ctxt.tile([128, CC, 512], BF, tag="ctxT", name="ctxT")
                nc.sync.dma_start_transpose(
                    out=ctxT[:, :, 0:bw], in_=ctx_d[base : base + bw, :]
                )
                return ctxT

            def emit_k_block(bi, ctxT):
                base, bw = MBLK[bi]
                for dc in range(CI):
                    psk = ps_pv.tile([128, 512], F32, tag="pv", name="psk")
                    for c in range(CC):
                        nc.tensor.matmul(
                            psk[:, :bw],
                            wk[:, c, dc * 128 : (dc + 1) * 128],
                            ctxT[:, c, :bw],
                            start=(c == 0),
                            stop=(c == CC - 1),
                        )
                    nc.vector.tensor_copy(
                        kT8[:, dc // 2, dc % 2, base : base + bw], psk[:, :bw]
                    )

            def emit_v_ktile(bi, ktl, ctxT):
                base, bw = MBLK[bi]
                t = base // 128 + ktl
                psv = ps_pv.tile([128, 512], F32, tag="pv", name="psv")
                for c in range(CC):
                    nc.tensor.matmul(
                        psv[:],
                        ctxT[:, c, ktl * 128 : (ktl + 1) * 128],
                        wv[:, c, :],
                        start=(c == 0),
                        stop=(c == CC - 1),
                    )
                nc.vector.tensor_copy(
                    v2[:, t, :, 0:64],
                    psv[:].rearrange("p (h d) -> p h d", d=64),
                )

            def emit_v_block(bi, ctxT):
                base, bw = MBLK[bi]
                for ktl in range(bw // 128):
                    emit_v_ktile(bi, ktl, ctxT)

            # ---- attention unit pieces ----
            def unit_scores_group(qb, h, g0, gn, pb, tays=None):
                hi, b4 = h // 4, h % 4
                p0 = 32 * b4
                sc = ps_sc.tile([128, 1536], F32, tag="sc", name="sc")
                for j in range(gn):
                    t = g0 + j
                    nc.tensor.matmul(
                        sc[:, j * 512 : (j + 1) * 512],
                        kT8[p0 : p0 + 32, hi, :, t * 128 : (t + 1) * 128],
                        qT8[p0 : p0 + 32, hi, :, qb * QB : (qb + 1) * QB],
                        start=True,
                        stop=True,
                        perf_mode=DRM,
                        tile_position=(p0, 0),
                    )
                scv = sc[:, 0 : gn * 512].rearrange("p (g q) -> p g q", q=512)
                if tays is None:
                    nc.scalar.activation(pb[:, g0 : g0 + gn, :], scv, AF.Exp, scale=SCALE)
                else:
                    # offload pass 1 on Pool: t = ALPHA*s + BETA (fp16)
                    ta, _ = tays
                    nc.gpsimd.tensor_scalar(
                        ta[:, g0 : g0 + gn, :],
                        scv,
                        ALPHA,
                        BETA,
                        mybir.AluOpType.mult,
                        mybir.AluOpType.add,
                    )

            def unit_exp_offload_squarings(pb, tays):
                """(t^2+0.5)^16 in two half-unit passes on DVE."""
                ta, tb = tays
                for h0, h1 in HALF:
                    n = h1 - h0
                    a = ta[:, h0:h1, :]
                    b = tb[:, 0:n, :]
                    nc.vector.tensor_mul(b, a, a)  # v = t^2
                    nc.vector.tensor_scalar(
                        a, b, 1.0, 0.5, mybir.AluOpType.mult, mybir.AluOpType.add
                    )  # u = v + 0.5  (= w since w = t^2+0.5... shifted)
                    nc.vector.tensor_mul(b, a, a)  # w^2
                    nc.vector.tensor_mul(a, b, b)  # w^4
                    nc.vector.tensor_mul(b, a, a)  # w^8
                    nc.vector.tensor_mul(pb[:, h0:h1, :], b, b)  # w^16 -> bf16

            def unit_pv_chain(qb, h, c, pb, pv4):
                # all 4 chains of a unit share one PSUM bank at col c*65
                for t in range(T):
                    nc.tensor.matmul(
                        pv4[:, c * 65 : c * 65 + 65],
                        pb[:, t, c * 128 : (c + 1) * 128],
                        v2[:, t, h, :],
                        start=(t == 0),
                        stop=(t == T - 1),
                    )

            def unit_normalize(qb, h, pv4):
                pv4v = pv4[:, 0:260].rearrange("p (c e) -> p c e", e=65)
                linv = sm.tile([128, 4, 1], F32, tag="linv", name="linv")
                nc.vector.reciprocal(linv[:], pv4v[:, :, 64:65])
                nc.vector.tensor_mul(
                    onat[qb][:, :, h, :],
                    pv4v[:, :, 0:64],
                    linv[:].to_broadcast([128, 4, 64]),
                )

            def emit_oT_pair(qb, hp):
                # transpose one head-pair's slice of onat as soon as both
                # heads' normalize is done; stage it contiguous on DVE so a
                # SINGLE crossbar transpose covers all 4 q-subtiles (one
                # 625ns HWDGE stage instead of four)
                stgt = sm.tile([128, 4, 2, 64], BF, tag="otst", name="otst")
                nc.vector.tensor_copy(stgt[:], onat[qb][:, :, 2 * hp : 2 * hp + 2, :])
                nc.sync.dma_start_transpose(
                    out=oT[qb][:, hp, :].rearrange("p (t q) -> p t q", q=128),
                    in_=stgt[:],
                )

            def out_proj_chain(qb, qtl):
                pso = ps_pv.tile([128, 512], F32, tag="pv", name="pso")
                for c in range(CI):
                    nc.tensor.matmul(
                        pso[:],
                        oT[qb][:, c, qtl * 128 : (qtl + 1) * 128],
                        wo[:, c, :],
                        start=(c == 0),
                        stop=(c == CI - 1),
                    )
                ot = outp.tile([128, QUERY_DIM], BF, tag="ot", name="ot")
                nc.vector.tensor_add(ot[:], pso[:], bo_bc[:])
                qt = qb * (QB // 128) + qtl
                nc.sync.dma_start(out=out_d[qt * 128 : (qt + 1) * 128, :], in_=ot[:])

            # ---- emission schedule (software pipeline) ----
            UNITS = [(qb, h) for qb in range(NQB) for h in range(H)]
            NU = len(UNITS)
            PRO = PBUFS  # units whose scores interleave into the prologue
            pbs = {}
            tayss = {}
            done_g = {}  # unit -> next un-emitted score-group index

            def new_unit_bufs(ui):
                pbs[ui] = pbp.tile([128, T, 512], BF, tag="pb", name="pb")
                if ui in dve_units:
                    tayss[ui] = (
                        tay.tile([128, T, 512], FP16, tag="ta", name="ta"),
                        tay.tile([128, TH, 512], FP16, tag="tb", name="tb"),
                    )
                else:
                    tayss[ui] = None

            for ui in range(PRO):
                new_unit_bufs(ui)
                done_g[ui] = 0

            # ctxT0 transpose queued right behind wk, before the cold-path DMAs
            ctxT0 = emit_ctx_block(0)
            ctxT1_pre = emit_ctx_block(1)
            nc.sync.dma_start_transpose(out=xT[:, :, 512:1024], in_=xs_d[512:1024, :])
            nc.gpsimd.dma_start(
                out=wv[:], in_=wv_d[:].rearrange("(o p) f -> p o f", p=128)
            )
            ctxT2_pre = emit_ctx_block(2)
            ctxT3_pre = emit_ctx_block(3)
            ctxT4_pre = emit_ctx_block(4)
            nc.gpsimd.dma_start(
                out=wo[:], in_=wo_d[:].rearrange("(o p) f -> p o f", p=128)
            )
            nc.sync.dma_start(
                out=bo_bc[:],
                in_=bass.AP(tensor=bo_d, offset=0, ap=[[0, 128], [1, QUERY_DIM]]),
            )
            # valid column of v2 straight from DRAM (one DMA per head):
            # v2[p, t, h, 64] = valid[t*128 + p]
            for h in range(H):
                nc.sync.dma_start(
                    out=v2[:, :, h, 64:65],
                    in_=bass.AP(tensor=val_d, offset=0, ap=[[1, 128], [128, T]]),
                )

            # PE warmup: keep the tensor engine continuously busy from t=0
            # so the p-state ramp reaches full clock before Q proj; matmuls on
            # a zeroed tile, results never read.
            wz = big.tile([128, 640], BF, tag="wz", name="wz")
            nc.vector.memset(wz[:], 0.0)
            for _ in range(WARMUP_MM):
                psw = ps_pv.tile([128, 512], F32, tag="pv", name="psw")
                nc.tensor.matmul(
                    psw[:], wz[:, 0:128], wz[:, 128:640], start=True, stop=True
                )

            # Q proj: just the two chunks units h0-3 need, so the first
            # score group (and ACT) starts as early as possible
            emit_q_proj_qf(0, [0, 1])
            # keep the PE clock hot while wk/ctxT0 land
            for _ in range(WARMUP2_MM):
                psw = ps_pv.tile([128, 512], F32, tag="pv", name="psw")
                nc.tensor.matmul(
                    psw[:], wz[:, 0:128], wz[:, 128:640], start=True, stop=True
                )
            q_hi1 = [False]

            # ---- prologue as a woven stream: block-0 K inline, then score
            # groups of the PRO units alternating with deferred PE work
            # (remaining Q chunks, V0, Q qf1, and the NEXT blocks' K chains)
            # so ACT never waits a whole K-block.
            emit_k_block(0, ctxT0)
            tiles_emitted = [MBLK[0][1] // 128]
            groups_pending = []

            def unlock():
                tr = tiles_emitted[0]
                for ui in range(PRO):
                    qb, h = UNITS[ui]
                    if h >= 4 and not q_hi1[0]:
                        continue
                    while (
                        done_g[ui] < len(GROUPS)
                        and GROUPS[done_g[ui]][0] + GROUPS[done_g[ui]][1] <= tr
                    ):
                        g0, gn = GROUPS[done_g[ui]]
                        groups_pending.append((ui, g0, gn))
                        done_g[ui] += 1

            unlock()

            pro_w = []

            def _qdc23():
                emit_q_proj_qf(0, [2, 3])
                q_hi1[0] = True
                unlock()

            pro_w.append(_qdc23)

            pro_ctx = {0: ctxT0, 1: ctxT1_pre, 2: ctxT2_pre, 3: ctxT3_pre,
                       4: ctxT4_pre}

            def mk_kchain(b, dc):
                def f():
                    if dc == 0 and b not in pro_ctx:
                        pro_ctx[b] = emit_ctx_block(b)
                    base, bw = MBLK[b]
                    psk = ps_pv.tile([128, 512], F32, tag="pv", name="psk")
                    for c in range(CC):
                        nc.tensor.matmul(
                            psk[:, :bw],
                            wk[:, c, dc * 128 : (dc + 1) * 128],
                            pro_ctx[b][:, c, :bw],
                            start=(c == 0),
                            stop=(c == CC - 1),
                        )
                    nc.vector.tensor_copy(
                        kT8[:, dc // 2, dc % 2, base : base + bw], psk[:, :bw]
                    )
                    if dc == CI - 1:
                        tiles_emitted[0] += bw // 128
                        unlock()
                return f

            deferred_v = []
            for b in range(1, len(MBLK)):
                for dc in range(CI):
                    pro_w.append(mk_kchain(b, dc))
                deferred_v.append(b)
                if b == 1:
                    # V0 and Q qf1 wait on the cold DMAs (wv, xT half 2);
                    # schedule them after block 1's K chains so they don't
                    # head-of-line-block the score stream
                    pro_w.append(lambda: emit_v_block(0, ctxT0))
                    for dc in range(CI):
                        pro_w.append(lambda dc=dc: emit_q_proj_qf(1, [dc]))
            # deferred V k-tiles weave into the prologue tail (ACT has the
            # block-3/4 group backlog there), emptying the steady-entry
            # guard burst
            for bi in deferred_v:
                base, bw = MBLK[bi]
                for ktl in range(bw // 128):
                    pro_w.append(
                        lambda bi=bi, ktl=ktl: emit_v_ktile(bi, ktl, pro_ctx[bi])
                    )

            wi = [0]
            while groups_pending or wi[0] < len(pro_w):
                if groups_pending:
                    ui, g0, gn = groups_pending.pop(0)
                    qb, h = UNITS[ui]
                    unit_scores_group(qb, h, g0, gn, pbs[ui], tayss[ui])
                if wi[0] < len(pro_w):
                    pro_w[wi[0]]()
                    wi[0] += 1

            for ui in range(PRO):
                if tayss[ui] is not None:
                    unit_exp_offload_squarings(pbs[ui], tayss[ui])

            # ---- steady state: score emission paced against a global filler
            # queue of PV chains / normalizes / out-proj work. Pacing target:
            # by the end of unit fu's score groups the filler has emitted all
            # of unit fu's own PV chains, so PV trails scores by < 1 unit and
            # nothing piles up after the last exp.
            pv4s = {}

            def mk_chain(u, qb, h, c):
                def f():
                    if c == 0:
                        pv4s[u] = ps_p4.tile([128, 512], F32, tag="pv4", name="pv4")
                    unit_pv_chain(qb, h, c, pbs[u], pv4s[u])
                    if c == 3:
                        unit_normalize(qb, h, pv4s[u])
                        if h % 2 == 1:
                            emit_oT_pair(qb, h // 2)
                return f

            flat_fill = []
            chain_end = {}
            for u in range(NU):
                qb, h = UNITS[u]
                for c in range(4):
                    flat_fill.append(mk_chain(u, qb, h, c))
                chain_end[u] = len(flat_fill)
                if h == H - 1:
                    for qtl in range(CI):
                        flat_fill.append(
                            lambda qb=qb, qtl=qtl: out_proj_chain(qb, qtl)
                        )

            fi = [0]

            def pump_to(target):
                while fi[0] < min(target, len(flat_fill)):
                    flat_fill[fi[0]]()
                    fi[0] += 1

            NG = len(GROUPS)
            NFILL = len(flat_fill)
            NSTEADY = NU - PRO
            for fu in range(PRO, NU):
                fqb, fh = UNITS[fu]
                # make sure the pb ring slot this unit reuses is fully retired
                pump_to(chain_end[fu - PBUFS] if fu >= PBUFS else 0)
                new_unit_bufs(fu)
                for gi, (g0, gn) in enumerate(GROUPS):
                    unit_scores_group(fqb, fh, g0, gn, pbs[fu], tayss[fu])
                    # Tile deps are emission-ordered: a PV chain of unit u may
                    # only be emitted once ALL of u's exps are emitted, i.e.
                    # u <= fu-1 while unit fu's groups are in flight.
                    frac = (fu - PRO + (gi + 1) / NG) / (NSTEADY + 1)
                    pump_to(min(chain_end[fu - 1], int(frac * NFILL + 0.5), fi[0] + 2))
                if tayss[fu] is not None:
                    unit_exp_offload_squarings(pbs[fu], tayss[fu])
            pump_to(NFILL)
            if dbg:
                nc.sync.dma_start(out=dq_d[:], in_=qT8[:])
                nc.sync.dma_start(out=dk_d[:], in_=kT8[:])
                nc.sync.dma_start(out=dv_d[:], in_=v2[:])
                nc.sync.dma_start(out=don_d[:], in_=onat[0][:])
                nc.sync.dma_start(out=dot_d[:], in_=oT[0][:])
                nc.sync.dma_start(out=don1_d[:], in_=onat[1][:])
                nc.sync.dma_start(out=dot1_d[:], in_=oT[1][:])
                nc.sync.dma_start(out=dpb_d[:], in_=pbs[15][:])

    nc.compile()
    return nc


def kernel(x, context_tensor, mask, Wq, Wk, Wv, Wo, bo):
    import ml_dtypes
    from concourse.bass_utils import run_bass_kernel_spmd

    BFnp = ml_dtypes.bfloat16
    x = np.asarray(x, dtype=np.float32)
    context_tensor = np.asarray(context_tensor, dtype=np.float32)
    mask = np.asarray(mask)
    perm = _perm()
    Wq = np.asarray(Wq, dtype=np.float32)[:, perm].astype(BFnp)
    Wk = np.asarray(Wk, dtype=np.float32)[:, perm].astype(BFnp)
    Wv = np.asarray(Wv, dtype=np.float32).astype(BFnp)
    Wo = np.asarray(Wo, dtype=np.float32).astype(BFnp)
    bo = np.ascontiguousarray(np.asarray(bo, dtype=np.float32))

    # host-side context compaction using the mask
    meffs = [int(mask[b].sum()) for b in range(B)]
    m_eff = max(max(meffs), 1)
    m_pad = max(((m_eff + 127) // 128) * 128, M_PAD_MIN)
    ctx_c = np.zeros((B, m_pad, CONTEXT_DIM), dtype=BFnp)
    val = np.zeros((B, m_pad), dtype=BFnp)
    for b in range(B):
        idx = np.flatnonzero(mask[b])
        ctx_c[b, : len(idx)] = context_tensor[b, idx].astype(BFnp)
        val[b, : len(idx)] = 1.0
    xb = x.astype(BFnp)

    if m_pad not in _compiled:
        _compiled[m_pad] = _build(m_pad)
    nc = _compiled[m_pad]

    rows_per_core = N // (NCORES // B)  # 1024
    in_maps = []
    for d in range(NCORES):
        b = d // (NCORES // B)
        r0 = (d % (NCORES // B)) * rows_per_core
        in_maps.append(
            {
                "xs": xb[b, r0 : r0 + rows_per_core],
                "ctx": ctx_c[b],
                "valid": val[b],
                "Wq": Wq,
                "Wk": Wk,
                "Wv": Wv,
                "Wo": Wo,
                "bo": bo,
            }
        )

    res = run_bass_kernel_spmd(nc, in_maps, list(range(NCORES)))
    out = np.empty((B, N, QUERY_DIM), dtype=np.float32)
    for d in range(NCORES):
        b = d // (NCORES // B)
        r0 = (d % (NCORES // B)) * rows_per_core
        out[b, r0 : r0 + rows_per_core] = res.results[d]["out"].astype(np.float32)
    return out
